# revision 1
# baseline (speedup 1.0000x reference)
"""Trainium2 Bass kernel for nn_MultiHeadAttention (BS=4, L=2048, D=1024, H=16).

Sharding: 8 cores = 4 batches x 2 query-halves. Each core computes attention
for 1024 query rows of one batch over all 16 heads, with K/V restricted to
that batch's unmasked key positions (host-side gather; masked keys contribute
exactly 0 to the reference softmax, and padding slots get bias -1e30 so
exp() makes them exactly 0 on device too). No cross-core communication; the
host concatenates the 8 [1024, 1024] outputs.

Per-core dataflow (everything kept transposed / d-major so no on-device
transposes are ever needed):
  A3: V[tok, dv]   = (xkv^T)^T @ Wv + bv      (stationary = xkv^T tiles)
      stored with a ones-column appended per head (65 cols/head) so that the
      PV matmul also produces the softmax denominator in its 65th row.
  A2: K^T[dv, tok] = Wk^T-tiles @ xkv^T + bk
  A1: Q^T[dv, tok] = Wq^T-tiles @ xq^T + bq
  B:  per (query-chunk qc of 512, head-pair p):
        S^T[k,q] via row-packed K=64 matmul pairs (2 heads concurrently on
        the PE via partition offsets 0/64) -> 2-bank PSUM [128, 1024]
        exp via one ScalarE ACTIVATE (scale=1/8, bias = mask column) -> P^T
        O^T for both heads + their denominators accumulate into ONE 2-bank
        PSUM tile o2 [128,1024] (rows 0:64 = O^T, row 64 = sums) via M=65
        PV matmuls; then per pair: reciprocal_approx_fast directly on the
        PSUM sums row, bf16 cast, two K=1 col-tiled broadcast matmuls fan
        the reciprocals across partitions into free PSUM rows of o2, and
        two DVE multiplies normalize o_t in place (one PSUM operand each).
  C:  out[q, n] = o_t-tiles^T @ Wf + bf, streamed out per 128-row tile
      (bf16 output, PSUM evicted on alternating Scalar/Vector engines).

Matmul operands are bf16 with N moving where PSUM allows; accumulation is
fp32 in PSUM. Emission starts attention as early as possible (a3, then the
first two head-pairs' K/Q projections) so the ScalarE exp stream - the
second-longest engine chain - is hidden under PE work.
"""
import sys

sys.path.insert(0, "/opt/trn_rl_repo")

import numpy as np
import ml_dtypes

BF16NP = ml_dtypes.bfloat16

BS, L, D, H, DK = 4, 2048, 1024, 16, 64
SCALE = 1.0 / np.sqrt(DK)
QH = L // 2          # queries per core
NP = H // 2          # head pairs
MT = D // 128        # out-dim tiles
DT = D // 128        # contraction d tiles
NEG = -1.0e30

_programs = {}
_ONES = np.ones((128, 128), BF16NP)


def _chunks(total, maxc=512):
    n = -(-total // maxc)
    base = total // n
    rem = total - base * n
    return [base + (1 if i < rem else 0) for i in range(n)]


def _build(KC, with_bias=True):
    import contextlib

    import concourse.bacc as bacc
    import concourse.bass as bass
    import concourse.tile as tile
    import concourse.mybir as mybir

    F32 = mybir.dt.float32
    BF16 = mybir.dt.bfloat16
    AF = mybir.ActivationFunctionType
    ts = bass.ts

    KCT = KC // 128
    kch = _chunks(KC)

    nc = bacc.Bacc("TRN2", target_bir_lowering=False)

    xq_d = nc.dram_tensor("xq", [D, QH], BF16, kind="ExternalInput")
    xkv_d = nc.dram_tensor("xkv", [D, KC], BF16, kind="ExternalInput")
    wq_d = nc.dram_tensor("wq", [D, D], BF16, kind="ExternalInput")
    wk_d = nc.dram_tensor("wk", [D, D], BF16, kind="ExternalInput")
    wv_d = nc.dram_tensor("wv", [D, D], BF16, kind="ExternalInput")
    wf_d = nc.dram_tensor("wf", [D, D], BF16, kind="ExternalInput")
    mb_d = nc.dram_tensor("mb", [128, KCT], F32, kind="ExternalInput")
    ones_d = nc.dram_tensor("ones", [128, 128], BF16, kind="ExternalInput")
    bq_d = nc.dram_tensor("bqt", [128, MT], F32, kind="ExternalInput")
    bk_d = nc.dram_tensor("bkt", [128, MT], F32, kind="ExternalInput")
    bv_d = nc.dram_tensor("bvr", [1, D], BF16, kind="ExternalInput")
    bf_d = nc.dram_tensor("bfr", [1, D], BF16, kind="ExternalInput")
    out_d = nc.dram_tensor("out", [QH, D], BF16, kind="ExternalOutput")

    with tile.TileContext(nc) as tc, nc.allow_low_precision(
        reason="bf16 matmul pipeline with fp32 accumulation"
    ), contextlib.ExitStack() as ctx:
        const = ctx.enter_context(tc.tile_pool(name="const", bufs=1))
        persist = ctx.enter_context(tc.tile_pool(name="persist", bufs=1))
        ppool = ctx.enter_context(tc.tile_pool(name="ppool", bufs=4))
        rpool = ctx.enter_context(tc.tile_pool(name="rpool", bufs=2))
        bcpool = ctx.enter_context(tc.tile_pool(name="bcpool", bufs=2))
        outpool = ctx.enter_context(tc.tile_pool(name="outpool", bufs=2))
        psum = ctx.enter_context(tc.tile_pool(name="psum", bufs=2, space="PSUM"))
        drpool = ctx.enter_context(tc.tile_pool(name="drpool", bufs=2,
                                                space="DRAM"))

        ones128 = const.tile([128, 128], BF16, name="ones128")
        nc.sync.dma_start(ones128[:], ones_d[:])
        mb_sb = const.tile([128, KCT], F32, name="mb_sb")
        nc.sync.dma_start(mb_sb[:], mb_d[:])
        bq_sb = const.tile([128, MT], F32, name="bq_sb")
        nc.sync.dma_start(bq_sb[:], bq_d[:])
        bk_sb = const.tile([128, MT], F32, name="bk_sb")
        nc.sync.dma_start(bk_sb[:], bk_d[:])
        bv_sb = const.tile([1, D], BF16, name="bv_sb")
        nc.sync.dma_start(bv_sb[:], bv_d[:])
        bf_sb = const.tile([1, D], BF16, name="bf_sb")
        nc.sync.dma_start(bf_sb[:], bf_d[:])

        q_t = [persist.tile([128, QH], BF16, name=f"q_t{m}", tag=f"q_t{m}")
               for m in range(MT)]
        k_t = [persist.tile([128, KC], BF16, name=f"k_t{m}", tag=f"k_t{m}")
               for m in range(MT)]
        v65 = [persist.tile([128, H * 65], BF16, name=f"v65_{t}", tag=f"v65_{t}")
               for t in range(KCT)]
        o_t = [persist.tile([128, QH], BF16, name=f"o_t{p}", tag=f"o_t{p}")
               for p in range(NP)]
        xkv = [persist.tile([128, KC], BF16, name=f"xkv{d}", tag=f"xkv{d}")
               for d in range(DT)]
        xq = [persist.tile([128, QH], BF16, name=f"xq{d}", tag=f"xq{d}")
              for d in range(DT)]
        wv = [persist.tile([128, D], BF16, name=f"wv{d}", tag=f"wv{d}")
              for d in range(DT)]
        wk = [persist.tile([128, D], BF16, name=f"wk{d}", tag=f"wk{d}")
              for d in range(DT)]
        wq = [persist.tile([128, D], BF16, name=f"wq{d}", tag=f"wq{d}")
              for d in range(DT)]
        wf = [persist.tile([128, D], BF16, name=f"wf{d}", tag=f"wf{d}")
              for d in range(DT)]

        # first key-column block + wv unblock a3's first tiles ~3us sooner
        for d in range(DT):
            nc.sync.dma_start(xkv[d][:, 0:512], xkv_d[ts(d, 128), 0:512])
            nc.sync.dma_start(wv[d][:], wv_d[ts(d, 128), :])
        for d in range(DT):
            nc.sync.dma_start(xkv[d][:, 512:KC], xkv_d[ts(d, 128), 512:KC])
        for t in range(KCT):
            v3 = v65[t].rearrange("p (h e) -> p h e", e=65)
            nc.vector.tensor_copy(
                v3[:, :, 64:65],
                ones128[:, 0:H].rearrange("p (h e) -> p h e", e=1))

        # ---------------- phase pieces ----------------
        def a3(trange):
            # V (k-major) with ones column interleaved; N=1024 moving
            for t in trange:
                pv = psum.tile([128, 1024], F32, name=f"pv{t}", tag="ps1024")
                for d in range(DT):
                    for c in range(2):
                        nc.tensor.matmul(pv[:, ts(c, 512)],
                                         xkv[d][:, ts(t, 128)],
                                         wv[d][:, ts(c, 512)],
                                         start=(d == 0),
                                         stop=(not with_bias and d == DT - 1))
                if with_bias:
                    for c in range(2):
                        nc.tensor.matmul(pv[:, ts(c, 512)], ones128[0:1, 0:128],
                                         bv_sb[0:1, ts(c, 512)],
                                         start=False, stop=True)
                dst = v65[t].rearrange("p (h e) -> p h e", e=65)
                src = pv.rearrange("p (h e) -> p h e", e=64)
                # pre-attention window: ScalarE is idle, keep DVE free
                nc.scalar.copy(dst[:, :, 0:64], src[:])

        def a2(mrange, scalar_evict=False):
            for m in mrange:
                pk = [psum.tile([128, w], F32, name=f"pk{m}_{c}", tag="ps1024")
                      for c, w in enumerate(kch)]
                for d in range(DT):
                    off = 0
                    for c, w in enumerate(kch):
                        nc.tensor.matmul(
                            pk[c][:], wk[d][:, ts(m, 128)],
                            xkv[d][:, off:off + w],
                            start=(d == 0), stop=(d == DT - 1))
                        off += w
                off = 0
                for c, w in enumerate(kch):
                    dst = k_t[m][:, off:off + w]
                    if with_bias:
                        nc.vector.tensor_scalar_add(dst, pk[c][:],
                                                    bk_sb[:, m:m + 1])
                    elif scalar_evict:
                        nc.scalar.copy(dst, pk[c][:])
                    else:
                        nc.vector.tensor_copy(dst, pk[c][:])
                    off += w

        def a1(mrange, scalar_evict=False):
            for m in mrange:
                pq = psum.tile([128, 1024], F32, name=f"pq{m}", tag="ps1024")
                for d in range(DT):
                    for c in range(2):
                        nc.tensor.matmul(pq[:, ts(c, 512)],
                                         wq[d][:, ts(m, 128)],
                                         xq[d][:, ts(c, 512)],
                                         start=(d == 0), stop=(d == DT - 1))
                if with_bias:
                    nc.vector.tensor_scalar_add(q_t[m][:], pq[:],
                                                bq_sb[:, m:m + 1])
                elif scalar_evict:
                    nc.scalar.copy(q_t[m][:], pq[:])
                else:
                    nc.vector.tensor_copy(q_t[m][:], pq[:])

        pending_norm = []

        def b_pairs(qc, prange):
            prs = list(prange)
            for i in range(0, len(prs), 2):
                grp = prs[i:i + 2]
                oacc = {}
                for p in grp:
                    # one 2-bank accumulator per pair: cols 0:512 head A,
                    # 512:1024 head B; row 64 = softmax denominators
                    oacc[p] = psum.tile([128, 1024], F32, name=f"o2_{qc}_{p}",
                                        tag="oB", bufs=2)
                for kt in range(KCT):
                    pps = {}
                    for p in grp:
                        s = psum.tile([128, 1024], F32,
                                      name=f"s_{qc}_{p}_{kt}", tag="ps1024")
                        nc.tensor.matmul(
                            s[:, 0:512], k_t[p][0:64, ts(kt, 128)],
                            q_t[p][0:64, ts(qc, 512)])
                        nc.tensor.matmul(
                            s[:, 512:1024], k_t[p][64:128, ts(kt, 128)],
                            q_t[p][64:128, ts(qc, 512)])
                        pp = ppool.tile([128, 1024], BF16,
                                        name=f"pp_{qc}_{p}_{kt}", tag="pp")
                        nc.scalar.activation(
                            pp[:], s[:], AF.Exp,
                            bias=mb_sb[:, kt:kt + 1], scale=float(SCALE))
                        pps[p] = pp
                    for p in grp:
                        h0, h1 = 2 * p, 2 * p + 1
                        o2 = oacc[p]
                        nc.tensor.matmul(
                            o2[0:65, 0:512], v65[kt][:, h0 * 65:(h0 + 1) * 65],
                            pps[p][:, 0:512],
                            start=(kt == 0), stop=(kt == KCT - 1))
                        nc.tensor.matmul(
                            o2[0:65, 512:1024],
                            v65[kt][:, h1 * 65:(h1 + 1) * 65],
                            pps[p][:, 512:1024],
                            start=(kt == 0), stop=(kt == KCT - 1))
                # stage the sums rows to SBUF and evacuate both heads
                # immediately so the PSUM slots free fast; the longer
                # reciprocal/broadcast/multiply chain is emitted later
                # (b_norm_late) so it cannot starve PSUM evictions of the
                # interleaved projection work on the DVE queue
                dens = []
                for p in grp:
                    o2 = oacc[p]
                    qsl = ts(qc, 512)
                    den = rpool.tile([1, 1024], F32, name=f"den_{qc}_{p}",
                                     tag="den", bufs=3)
                    nc.vector.tensor_copy(den[:], o2[64:65, :])
                    nc.vector.tensor_copy(o_t[p][0:64, qsl], o2[0:64, 0:512])
                    nc.vector.tensor_copy(o_t[p][64:128, qsl],
                                          o2[0:64, 512:1024])
                    dens.append(den)
                pending_norm.append((qc, tuple(grp), dens))

        def b_norm_late():
            # 1/denominators (custom-DVE op needs SBUF input on HW), bf16,
            # broadcast across partitions via a DRAM bounce with 0-stride
            # reads; normalize o_t in place on the otherwise-idle GpSimd
            while pending_norm:
                qc, grp, dens = pending_norm.pop(0)
                for r, p in enumerate(grp):
                    qsl = ts(qc, 512)
                    rec = rpool.tile([1, 1024], F32, name=f"rec_{qc}_{p}",
                                     tag="rec")
                    nc.vector.reciprocal_approx_fast(rec[:], dens[r][:])
                    rdr = drpool.tile([1, 1024], F32, name=f"rdr_{qc}_{p}",
                                      tag="rdr")
                    nc.sync.dma_start(rdr[:], rec[:])
                    bcs = bcpool.tile([128, 512], F32,
                                      name=f"bcs_{qc}_{p}", tag="bcs")
                    nc.sync.dma_start(
                        bcs[0:64, :],
                        rdr[0:1, 0:512].to_broadcast([64, 512]))
                    nc.sync.dma_start(
                        bcs[64:128, :],
                        rdr[0:1, 512:1024].to_broadcast([64, 512]))
                    nc.vector.tensor_mul(o_t[p][0:64, qsl],
                                         o_t[p][0:64, qsl], bcs[0:64, :])
                    nc.vector.tensor_mul(o_t[p][64:128, qsl],
                                         o_t[p][64:128, qsl],
                                         bcs[64:128, :])

        def c_fc(qtrange):
            for qt in qtrange:
                fp = psum.tile([128, 1024], F32, name=f"fp{qt}", tag="ps1024")
                for dt in range(DT):
                    for c in range(2):
                        nc.tensor.matmul(fp[:, ts(c, 512)],
                                         o_t[dt][:, ts(qt, 128)],
                                         wf[dt][:, ts(c, 512)],
                                         start=(dt == 0),
                                         stop=(not with_bias and dt == DT - 1))
                if with_bias:
                    for c in range(2):
                        nc.tensor.matmul(fp[:, ts(c, 512)], ones128[0:1, 0:128],
                                         bf_sb[0:1, ts(c, 512)],
                                         start=False, stop=True)
                ost = outpool.tile([128, 1024], BF16, name=f"ost{qt}",
                                   tag="ost")
                # ScalarE is the exp engine while qt<4 runs - keep it free
                if qt >= 4 and qt % 2 == 1:
                    nc.scalar.copy(ost[:], fp[:])
                else:
                    nc.vector.tensor_copy(ost[:], fp[:])
                nc.sync.dma_start(out_d[ts(qt, 128), :], ost[:])

        # ---------------- emission order ----------------
        a3(range(KCT))
        for d in range(DT):
            nc.sync.dma_start(wk[d][:], wk_d[ts(d, 128), :])
        a2(range(0, 2), scalar_evict=True)
        for d in range(DT):
            nc.sync.dma_start(xq[d][:], xq_d[ts(d, 128), :])
            nc.sync.dma_start(wq[d][:], wq_d[ts(d, 128), :])
        a1(range(0, 2), scalar_evict=True)
        for d in range(DT):
            nc.sync.dma_start(wf[d][:], wf_d[ts(d, 128), :])
        b_pairs(0, range(0, 2))
        a2(range(2, 4))
        a1(range(2, 4))
        b_norm_late()
        b_pairs(0, range(2, 4))
        a2(range(4, 6))
        a1(range(4, 6))
        b_norm_late()
        b_pairs(0, range(4, 6))
        a2(range(6, 8))
        a1(range(6, 8))
        b_norm_late()
        b_pairs(0, range(6, 8))
        b_pairs(1, range(0, 2))
        b_norm_late()
        c_fc(range(0, 1))
        b_pairs(1, range(2, 4))
        b_norm_late()
        c_fc(range(1, 2))
        b_pairs(1, range(4, 6))
        b_norm_late()
        c_fc(range(2, 3))
        b_pairs(1, range(6, 8))
        c_fc(range(3, 4))
        b_norm_late()
        c_fc(range(4, 8))

    nc.finalize()
    return nc


_LDW_PATCHED = False


def _enable_ldw_opt():
    global _LDW_PATCHED
    if _LDW_PATCHED:
        return
    import concourse.bass_utils as bu
    orig = bu.run_command

    def patched(cmd, *a, **k):
        cmd = [c.replace("--enable-ldw-opt=false", "--enable-ldw-opt=true")
               if isinstance(c, str) else c for c in cmd]
        return orig(cmd, *a, **k)

    bu.run_command = patched
    _LDW_PATCHED = True


def _get_program(KC, with_bias=True):
    key = (KC, with_bias)
    if key not in _programs:
        if __import__("os").environ.get("LDW_OPT"):
            _enable_ldw_opt()
        _programs[key] = _build(KC, with_bias)
    return _programs[key]


LAST_EXEC_NS = None
PROFILE = False


def _ensure_profile_hook():
    """Wire up the NTFF profile hook that the slim agent container leaves
    unconnected (antenv.axon_hooks is not injected; the ctypes hook body
    ships in trn_agent_boot)."""
    import types

    try:
        from antenv.axon_hooks import get_axon_ntff_profile_hook  # noqa: F401
        return
    except ImportError:
        pass
    import antenv

    mod = types.ModuleType("antenv.axon_hooks")
    _h = [None]
    mod.set_axon_ntff_profile_hook = lambda h: _h.__setitem__(0, h)
    mod.get_axon_ntff_profile_hook = lambda: _h[0]
    sys.modules["antenv.axon_hooks"] = mod
    antenv.axon_hooks = mod
    from trn_agent_boot.trn_boot import _ntff_profile_via_ctypes

    mod.set_axon_ntff_profile_hook(
        _ntff_profile_via_ctypes("/opt/axon/libaxon_pjrt.so"))
    # artifact upload needs a bucket this container doesn't have
    import concourse.bass_utils as bu

    bu.upload_artifacts = lambda tmpdir: f"local:{tmpdir}"


def kernel(x, mask, Wq, bq, Wk, bk, Wv, bv, Wf, bf):
    global LAST_EXEC_NS
    from concourse.bass_utils import run_bass_kernel_spmd

    if PROFILE:
        _ensure_profile_hook()

    x = np.asarray(x, dtype=np.float32)
    mask = np.asarray(mask)
    Wq16, Wk16, Wv16, Wf16 = (
        np.ascontiguousarray(np.asarray(w).astype(BF16NP))
        for w in (Wq, Wk, Wv, Wf))
    bq, bk = (np.asarray(v, np.float32) for v in (bq, bk))
    bv16, bf16v = (np.asarray(v).astype(BF16NP).reshape(1, D)
                   for v in (bv, bf))

    keeps = [np.nonzero(np.asarray(mask[b]) == 0)[0] for b in range(BS)]
    maxk = max(1, max(len(k) for k in keeps))
    KC = -(-maxk // 128) * 128
    with_bias = bool(np.any(np.asarray(bv)) or np.any(np.asarray(bf)))
    nc = _get_program(KC, with_bias)
    KCT = KC // 128

    bq_t = np.ascontiguousarray(bq.reshape(MT, 128).T)
    bk_t = np.ascontiguousarray(bk.reshape(MT, 128).T)

    x16 = x.astype(BF16NP)
    in_maps = []
    for c in range(8):
        b, j = divmod(c, 2)
        keep = keeps[b]
        xq_t = np.ascontiguousarray(x16[b, j * QH:(j + 1) * QH, :].T)
        xkv_t = np.zeros((D, KC), BF16NP)
        xkv_t[:, :len(keep)] = x16[b, keep, :].T
        mbv = np.full(KC, NEG, np.float32)
        mbv[:len(keep)] = 0.0
        mb_t = np.ascontiguousarray(mbv.reshape(KCT, 128).T)
        in_maps.append({
            "ones": _ONES, "xq": xq_t, "xkv": xkv_t,
            "wq": Wq16, "wk": Wk16, "wv": Wv16, "wf": Wf16,
            "mb": mb_t, "bqt": bq_t, "bkt": bk_t,
            "bvr": bv16, "bfr": bf16v,
        })

    res = run_bass_kernel_spmd(nc, in_maps, core_ids=list(range(8)),
                               trace=PROFILE)
    if res.exec_time_ns is not None:
        LAST_EXEC_NS = res.exec_time_ns

    out = np.empty((BS, L, D), np.float32)
    for c in range(8):
        b, j = divmod(c, 2)
        out[b, j * QH:(j + 1) * QH, :] = res.results[c]["out"]
    return out



# revision 2
# speedup vs baseline: 1.0004x; 1.0004x over previous
"""Trainium2 Bass kernel for nn_MultiHeadAttention (BS=4, L=2048, D=1024, H=16).

Sharding: 8 cores = 4 batches x 2 head-halves. Core (b, j) computes heads
8j..8j+8 of batch b for ALL 2048 queries, K/V over the batch's unmasked keys
(host gather). Wq/Wk/Wv column-sharded, Wf row-sharded; each core emits a
bf16 PARTIAL out = O_local @ Wf_rows and the host adds the two partials per
batch (+ bf).

Schedule: a3 (V), a2 (K), a1-half (Q cols 0:1024) run upfront, PE-dense.
The B phase processes one head-pair per group (16 groups of 9 key tiles);
it is exp-wall-bound on ScalarE (one ACTIVATE per key tile). PV matmuls lag
the S/exp stream by one key tile (software pipelining) so ScalarE never
starves, and the PE slack under the exp wall is filled by injecting the
remaining a1 matmuls and the first 8 FC (C) tiles one matmul per step
through the spare oB PSUM slot (one pair per group leaves one of the two oB
slots free to rotate through filler tiles). Denominator staging is a small
Vector copy; o_t normalization multiplies run on the otherwise-idle GpSimd
so the Vector FIFO cannot block PSUM evictions.
"""
import sys

sys.path.insert(0, "/opt/trn_rl_repo")

import numpy as np
import ml_dtypes

BF16NP = ml_dtypes.bfloat16

BS, L, D, H, DK = 4, 2048, 1024, 16, 64
SCALE = 1.0 / np.sqrt(DK)
QW = L             # queries per core
HL = 8             # local heads
NP = HL // 2       # local head pairs
MT = 4             # local out-dim tiles (512/128)
DT = D // 128      # contraction d tiles
NEG = -1.0e30

_programs = {}
_ONES = np.ones((128, 128), BF16NP)


def _chunks(total, maxc=512):
    n = -(-total // maxc)
    base = total // n
    rem = total - base * n
    return [base + (1 if i < rem else 0) for i in range(n)]


def _build(KC, with_bias=True):
    import contextlib

    import concourse.bacc as bacc
    import concourse.bass as bass
    import concourse.tile as tile
    import concourse.mybir as mybir

    F32 = mybir.dt.float32
    BF16 = mybir.dt.bfloat16
    AF = mybir.ActivationFunctionType
    ts = bass.ts

    KCT = KC // 128
    kch = _chunks(KC)

    nc = bacc.Bacc("TRN2", target_bir_lowering=False)

    xq_d = nc.dram_tensor("xq", [D, QW], BF16, kind="ExternalInput")
    xkv_d = nc.dram_tensor("xkv", [D, KC], BF16, kind="ExternalInput")
    wq_d = nc.dram_tensor("wq", [D, 512], BF16, kind="ExternalInput")
    wk_d = nc.dram_tensor("wk", [D, 512], BF16, kind="ExternalInput")
    wv_d = nc.dram_tensor("wv", [D, 512], BF16, kind="ExternalInput")
    wf_d = nc.dram_tensor("wf", [512, D], BF16, kind="ExternalInput")
    mb_d = nc.dram_tensor("mb", [128, KCT], F32, kind="ExternalInput")
    ones_d = nc.dram_tensor("ones", [128, 128], BF16, kind="ExternalInput")
    bq_d = nc.dram_tensor("bqt", [128, MT], F32, kind="ExternalInput")
    bk_d = nc.dram_tensor("bkt", [128, MT], F32, kind="ExternalInput")
    bv_d = nc.dram_tensor("bvr", [1, 512], BF16, kind="ExternalInput")
    out_d = nc.dram_tensor("out", [QW, D], BF16, kind="ExternalOutput")

    with tile.TileContext(nc) as tc, nc.allow_low_precision(
        reason="bf16 matmul pipeline with fp32 accumulation"
    ), contextlib.ExitStack() as ctx:
        const = ctx.enter_context(tc.tile_pool(name="const", bufs=1))
        persist = ctx.enter_context(tc.tile_pool(name="persist", bufs=1))
        ppool = ctx.enter_context(tc.tile_pool(name="ppool", bufs=4))
        rpool = ctx.enter_context(tc.tile_pool(name="rpool", bufs=3))
        bcpool = ctx.enter_context(tc.tile_pool(name="bcpool", bufs=4))
        outpool = ctx.enter_context(tc.tile_pool(name="outpool", bufs=2))
        psum = ctx.enter_context(tc.tile_pool(name="psum", bufs=2, space="PSUM"))
        drpool = ctx.enter_context(tc.tile_pool(name="drpool", bufs=3,
                                                space="DRAM"))

        q_t = [persist.tile([128, QW], BF16, name=f"q_t{m}", tag=f"q_t{m}")
               for m in range(MT)]
        k_t = [persist.tile([128, KC], BF16, name=f"k_t{m}", tag=f"k_t{m}")
               for m in range(MT)]
        v65 = [persist.tile([128, HL * 65], BF16, name=f"v65_{t}",
                            tag=f"v65_{t}") for t in range(KCT)]
        o_t = [persist.tile([128, QW], BF16, name=f"o_t{p}", tag=f"o_t{p}")
               for p in range(NP)]
        xkv = [persist.tile([128, KC], BF16, name=f"xkv{d}", tag=f"xkv{d}")
               for d in range(DT)]
        xq = [persist.tile([128, QW], BF16, name=f"xq{d}", tag=f"xq{d}")
              for d in range(DT)]
        wv = [persist.tile([128, 512], BF16, name=f"wv{d}", tag=f"wv{d}")
              for d in range(DT)]
        wk = [persist.tile([128, 512], BF16, name=f"wk{d}", tag=f"wk{d}")
              for d in range(DT)]
        wq = [persist.tile([128, 512], BF16, name=f"wq{d}", tag=f"wq{d}")
              for d in range(DT)]
        wf = [persist.tile([128, D], BF16, name=f"wf{d}", tag=f"wf{d}")
              for d in range(MT)]

        # a3 deps stream first: one full-KC DMA per xkv d-tile + wv
        for d in range(DT):
            nc.sync.dma_start(xkv[d][:], xkv_d[ts(d, 128), :])
            nc.sync.dma_start(wv[d][:], wv_d[ts(d, 128), :])
        # consts off the critical sync queue
        mb_sb = const.tile([128, KCT], F32, name="mb_sb")
        nc.scalar.dma_start(mb_sb[:], mb_d[:])
        ones128 = const.tile([128, 128], BF16, name="ones128")
        bq_sb = const.tile([128, MT], F32, name="bq_sb")
        bk_sb = const.tile([128, MT], F32, name="bk_sb")
        bv_sb = const.tile([1, 512], BF16, name="bv_sb")
        if with_bias:
            nc.scalar.dma_start(ones128[:], ones_d[:])
            nc.scalar.dma_start(bq_sb[:], bq_d[:])
            nc.scalar.dma_start(bk_sb[:], bk_d[:])
            nc.scalar.dma_start(bv_sb[:], bv_d[:])
        for t in range(KCT):
            v3 = v65[t].rearrange("p (h e) -> p h e", e=65)
            nc.vector.memset(v3[:, :, 64:65], 1.0)

        # ---------------- phase pieces ----------------
        def a3(trange):
            for t in trange:
                pv = psum.tile([128, 512], F32, name=f"pv{t}", tag="ps1024")
                for d in range(DT):
                    nc.tensor.matmul(pv[:], xkv[d][:, ts(t, 128)], wv[d][:],
                                     start=(d == 0),
                                     stop=(not with_bias and d == DT - 1))
                if with_bias:
                    nc.tensor.matmul(pv[:], ones128[0:1, 0:128], bv_sb[0:1, :],
                                     start=False, stop=True)
                dst = v65[t].rearrange("p (h e) -> p h e", e=65)
                src = pv.rearrange("p (h e) -> p h e", e=64)
                nc.scalar.copy(dst[:, :, 0:64], src[:])

        def a2(mrange, scalar_evict=False):
            for m in mrange:
                pk = [psum.tile([128, w], F32, name=f"pk{m}_{c}", tag="ps1024")
                      for c, w in enumerate(kch)]
                for d in range(DT):
                    off = 0
                    for c, w in enumerate(kch):
                        nc.tensor.matmul(
                            pk[c][:], wk[d][:, ts(m, 128)],
                            xkv[d][:, off:off + w],
                            start=(d == 0), stop=(d == DT - 1))
                        off += w
                off = 0
                for c, w in enumerate(kch):
                    dst = k_t[m][:, off:off + w]
                    if with_bias:
                        nc.vector.tensor_scalar_add(dst, pk[c][:],
                                                    bk_sb[:, m:m + 1])
                    elif scalar_evict:
                        nc.scalar.copy(dst, pk[c][:])
                    else:
                        nc.vector.tensor_copy(dst, pk[c][:])
                    off += w

        def a1_half(m, h2, scalar_evict=False, tag="ps1024"):
            # generator: one [128, 1024] query-column half of q_t[m];
            # yields after every matmul so it can hide in B-phase slack
            pq = psum.tile([128, 1024], F32, name=f"pq{m}_{h2}", tag=tag,
                           bufs=2)
            for d in range(DT):
                for c in range(2):
                    nc.tensor.matmul(
                        pq[:, ts(c, 512)], wq[d][:, ts(m, 128)],
                        xq[d][:, h2 * 1024 + c * 512:
                              h2 * 1024 + (c + 1) * 512],
                        start=(d == 0), stop=(d == DT - 1))
                    yield
            dst = q_t[m][:, h2 * 1024:(h2 + 1) * 1024]
            if with_bias:
                nc.vector.tensor_scalar_add(dst, pq[:], bq_sb[:, m:m + 1])
            elif scalar_evict:
                nc.scalar.copy(dst, pq[:])
            else:
                nc.vector.tensor_copy(dst, pq[:])
            yield

        def c_tile(qt, scalar_evict=False, tag="ps1024"):
            # generator: one [128 q, 1024] FC output tile
            fp = psum.tile([128, 1024], F32, name=f"fp{qt}", tag=tag, bufs=2)
            for dt in range(MT):
                for c in range(2):
                    nc.tensor.matmul(fp[:, ts(c, 512)],
                                     o_t[dt][:, ts(qt, 128)],
                                     wf[dt][:, ts(c, 512)],
                                     start=(dt == 0), stop=(dt == MT - 1))
                    yield
            ost = outpool.tile([128, 1024], BF16, name=f"ost{qt}", tag="ost")
            if scalar_evict:
                nc.scalar.copy(ost[:], fp[:])
            else:
                nc.vector.tensor_copy(ost[:], fp[:])
            nc.sync.dma_start(out_d[ts(qt, 128), :], ost[:])
            yield

        def run_all(gen):
            for _ in gen:
                pass

        pending_norm = []

        def b_group(qc, p, filler=None, fill_per_step=1):
            # one head pair x KCT key tiles; PV lags S/exp by one tile
            oacc = psum.tile([128, 1024], F32, name=f"o2_{qc}_{p}",
                             tag="oB", bufs=2)
            h0, h1 = 2 * p, 2 * p + 1

            def emit_pv(kt, pp):
                nc.tensor.matmul(
                    oacc[0:65, 0:512], v65[kt][:, h0 * 65:(h0 + 1) * 65],
                    pp[:, 0:512],
                    start=(kt == 0), stop=(kt == KCT - 1))
                nc.tensor.matmul(
                    oacc[0:65, 512:1024], v65[kt][:, h1 * 65:(h1 + 1) * 65],
                    pp[:, 512:1024],
                    start=(kt == 0), stop=(kt == KCT - 1))

            # PV lags the S/exp stream by LAG key tiles: keeps ScalarE fed
            # and gives the previous group's accumulator time to evict
            LAG = 2
            pps = {}
            for kt in range(KCT):
                s = psum.tile([128, 1024], F32, name=f"s_{qc}_{p}_{kt}",
                              tag="ps1024")
                nc.tensor.matmul(s[:, 0:512], k_t[p][0:64, ts(kt, 128)],
                                 q_t[p][0:64, ts(qc, 512)])
                nc.tensor.matmul(s[:, 512:1024], k_t[p][64:128, ts(kt, 128)],
                                 q_t[p][64:128, ts(qc, 512)])
                pp = ppool.tile([128, 1024], BF16, name=f"pp_{qc}_{p}_{kt}",
                                tag="pp")
                nc.scalar.activation(pp[:], s[:], AF.Exp,
                                     bias=mb_sb[:, kt:kt + 1],
                                     scale=float(SCALE))
                pps[kt] = pp
                if filler is not None:
                    for _ in range(fill_per_step):
                        try:
                            next(filler)
                        except StopIteration:
                            filler = None
                            break
                if kt >= LAG:
                    emit_pv(kt - LAG, pps.pop(kt - LAG))
            for kt in range(max(0, KCT - LAG), KCT):
                emit_pv(kt, pps.pop(kt))

            # stage denominator row + evacuate O^T (both heads)
            den = rpool.tile([1, 1024], F32, name=f"den_{qc}_{p}", tag="den",
                             bufs=3)
            nc.vector.tensor_copy(den[:], oacc[64:65, :])
            qsl = ts(qc, 512)
            nc.vector.tensor_copy(o_t[p][0:64, qsl], oacc[0:64, 0:512])
            nc.vector.tensor_copy(o_t[p][64:128, qsl], oacc[0:64, 512:1024])
            pending_norm.append((qc, p, den))
            return filler

        def b_norm_late():
            # reciprocal on DVE; broadcast via DRAM bounce; normalization
            # multiplies on the otherwise-idle GpSimd
            while pending_norm:
                qc, p, den = pending_norm.pop(0)
                rec = rpool.tile([1, 1024], F32, name=f"rec_{qc}_{p}",
                                 tag="rec")
                nc.vector.reciprocal_approx_fast(rec[:], den[:])
                rdr = drpool.tile([1, 1024], F32, name=f"rdr_{qc}_{p}",
                                  tag="rdr")
                nc.sync.dma_start(rdr[:], rec[:])
                qsl = ts(qc, 512)
                bcs = bcpool.tile([128, 512], F32, name=f"bcs_{qc}_{p}",
                                  tag="bcs")
                nc.sync.dma_start(bcs[0:64, :],
                                  rdr[0:1, 0:512].to_broadcast([64, 512]))
                nc.sync.dma_start(bcs[64:128, :],
                                  rdr[0:1, 512:1024].to_broadcast([64, 512]))
                nc.gpsimd.tensor_mul(o_t[p][0:64, qsl],
                                     o_t[p][0:64, qsl], bcs[0:64, :])
                nc.gpsimd.tensor_mul(o_t[p][64:128, qsl],
                                     o_t[p][64:128, qsl], bcs[64:128, :])

        def chain(*gens):
            for g in gens:
                yield from g

        # ---------------- emission order ----------------
        a3(range(KCT))
        for d in range(DT):
            nc.sync.dma_start(wk[d][:], wk_d[ts(d, 128), :])
        a2(range(0, 2), scalar_evict=True)
        for d in range(DT):
            nc.sync.dma_start(xq[d][:], xq_d[ts(d, 128), :])
            nc.sync.dma_start(wq[d][:], wq_d[ts(d, 128), :])
        a2(range(2, 4), scalar_evict=True)
        for m in range(MT):
            run_all(a1_half(m, 0, scalar_evict=True))
        for m in range(MT):
            nc.sync.dma_start(wf[m][:], wf_d[ts(m, 128), :])

        # B: 16 groups (qc x pair). Fillers ride the exp-wall slack through
        # the spare oB slot. a1 h1 must complete before qc2 (group 9);
        # c tiles for qc K become legal after b_norm_late of qc K.
        fillers = {
            (0, 0): a1_half(0, 1, tag="oB"),
            (0, 2): a1_half(1, 1, tag="oB"),
            (1, 0): a1_half(2, 1, tag="oB"),
            (1, 2): a1_half(3, 1, tag="oB"),
            (2, 0): chain(c_tile(0, tag="oB"), c_tile(1, tag="oB")),
            (2, 2): chain(c_tile(2, tag="oB"), c_tile(3, tag="oB")),
            (3, 0): chain(c_tile(4, tag="oB"), c_tile(5, tag="oB")),
            (3, 2): chain(c_tile(6, tag="oB"), c_tile(7, tag="oB")),
        }
        cur = None
        for qc in range(4):
            for p in range(NP):
                if (qc, p) in fillers:
                    cur = fillers[(qc, p)]
                cur = b_group(qc, p, filler=cur,
                              fill_per_step=2 if qc < 2 else 2)
            b_norm_late()
        for qt in range(8, 16):
            run_all(c_tile(qt, scalar_evict=True))

    nc.finalize()
    return nc


_LDW_PATCHED = False


def _enable_ldw_opt():
    global _LDW_PATCHED
    if _LDW_PATCHED:
        return
    import concourse.bass_utils as bu
    orig = bu.run_command

    def patched(cmd, *a, **k):
        cmd = [c.replace("--enable-ldw-opt=false", "--enable-ldw-opt=true")
               if isinstance(c, str) else c for c in cmd]
        return orig(cmd, *a, **k)

    bu.run_command = patched
    _LDW_PATCHED = True


def _get_program(KC, with_bias=True):
    key = (KC, with_bias)
    if key not in _programs:
        if __import__("os").environ.get("LDW_OPT"):
            _enable_ldw_opt()
        _programs[key] = _build(KC, with_bias)
    return _programs[key]


LAST_EXEC_NS = None
PROFILE = False


def _ensure_profile_hook():
    import types

    try:
        from antenv.axon_hooks import get_axon_ntff_profile_hook  # noqa: F401
        return
    except ImportError:
        pass
    import antenv

    mod = types.ModuleType("antenv.axon_hooks")
    _h = [None]
    mod.set_axon_ntff_profile_hook = lambda h: _h.__setitem__(0, h)
    mod.get_axon_ntff_profile_hook = lambda: _h[0]
    sys.modules["antenv.axon_hooks"] = mod
    antenv.axon_hooks = mod
    from trn_agent_boot.trn_boot import _ntff_profile_via_ctypes

    mod.set_axon_ntff_profile_hook(
        _ntff_profile_via_ctypes("/opt/axon/libaxon_pjrt.so"))
    import concourse.bass_utils as bu

    bu.upload_artifacts = lambda tmpdir: f"local:{tmpdir}"


def kernel(x, mask, Wq, bq, Wk, bk, Wv, bv, Wf, bf):
    global LAST_EXEC_NS
    from concourse.bass_utils import run_bass_kernel_spmd

    if PROFILE:
        _ensure_profile_hook()

    x = np.asarray(x, dtype=np.float32)
    mask = np.asarray(mask)
    Wq16, Wk16, Wv16, Wf16 = (
        np.ascontiguousarray(np.asarray(w).astype(BF16NP))
        for w in (Wq, Wk, Wv, Wf))
    bq32, bk32 = (np.asarray(v, np.float32) for v in (bq, bk))
    bv32 = np.asarray(bv, np.float32)
    bf32 = np.asarray(bf, np.float32)

    keeps = [np.nonzero(np.asarray(mask[b]) == 0)[0] for b in range(BS)]
    maxk = max(1, max(len(k) for k in keeps))
    KC = -(-maxk // 128) * 128
    with_bias = bool(np.any(bq32) or np.any(bk32) or np.any(bv32))
    nc = _get_program(KC, with_bias)
    KCT = KC // 128

    x16 = x.astype(BF16NP)
    in_maps = []
    for c in range(8):
        b, j = divmod(c, 2)
        keep = keeps[b]
        xq_t = np.ascontiguousarray(x16[b].T)
        xkv_t = np.zeros((D, KC), BF16NP)
        xkv_t[:, :len(keep)] = x16[b, keep, :].T
        mbv = np.full(KC, NEG, np.float32)
        mbv[:len(keep)] = 0.0
        mb_t = np.ascontiguousarray(mbv.reshape(KCT, 128).T)
        sl = slice(512 * j, 512 * (j + 1))
        in_maps.append({
            "ones": _ONES, "xq": xq_t, "xkv": xkv_t,
            "wq": np.ascontiguousarray(Wq16[:, sl]),
            "wk": np.ascontiguousarray(Wk16[:, sl]),
            "wv": np.ascontiguousarray(Wv16[:, sl]),
            "wf": np.ascontiguousarray(Wf16[sl, :]),
            "mb": mb_t,
            "bqt": np.ascontiguousarray(bq32[sl].reshape(MT, 128).T),
            "bkt": np.ascontiguousarray(bk32[sl].reshape(MT, 128).T),
            "bvr": bv32[sl].astype(BF16NP).reshape(1, 512),
        })

    res = run_bass_kernel_spmd(nc, in_maps, core_ids=list(range(8)),
                               trace=PROFILE)
    if res.exec_time_ns is not None:
        LAST_EXEC_NS = res.exec_time_ns

    out = np.empty((BS, L, D), np.float32)
    for b in range(BS):
        out[b] = (res.results[2 * b]["out"].astype(np.float32)
                  + res.results[2 * b + 1]["out"].astype(np.float32))
    out += bf32.reshape(1, 1, D)
    return out


# revision 3
# speedup vs baseline: 1.0139x; 1.0134x over previous
"""Trainium2 Bass kernel for nn_MultiHeadAttention (BS=4, L=2048, D=1024, H=16).

Sharding: 8 cores = 4 batches x 2 head-halves. Core (b, j) computes heads
8j..8j+8 of batch b for ALL 2048 queries, K/V over the batch's unmasked keys
(host gather). Wq/Wk/Wv column-sharded, Wf row-sharded; each core emits a
bf16 PARTIAL out = O_local @ Wf_rows and the host adds the two partials per
batch (+ bf).

Schedule: a3 (V), a2 (K), a1-half (Q cols 0:1024) run upfront, PE-dense.
The B phase processes one head-pair per group (16 groups of 9 key tiles);
it is exp-wall-bound on ScalarE (one ACTIVATE per key tile). PV matmuls lag
the S/exp stream by one key tile (software pipelining) so ScalarE never
starves, and the PE slack under the exp wall is filled by injecting the
remaining a1 matmuls and the first 8 FC (C) tiles one matmul per step
through the spare oB PSUM slot (one pair per group leaves one of the two oB
slots free to rotate through filler tiles). Denominator staging is a small
Vector copy; o_t normalization multiplies run on the otherwise-idle GpSimd
so the Vector FIFO cannot block PSUM evictions.
"""
import sys

sys.path.insert(0, "/opt/trn_rl_repo")

import numpy as np
import ml_dtypes

BF16NP = ml_dtypes.bfloat16

BS, L, D, H, DK = 4, 2048, 1024, 16, 64
SCALE = 1.0 / np.sqrt(DK)
QW = L             # queries per core
HL = 8             # local heads
NP = HL // 2       # local head pairs
MT = 4             # local out-dim tiles (512/128)
DT = D // 128      # contraction d tiles
NEG = -1.0e30

_programs = {}
_ONES = np.ones((128, 128), BF16NP)


def _chunks(total, maxc=512):
    n = -(-total // maxc)
    base = total // n
    rem = total - base * n
    return [base + (1 if i < rem else 0) for i in range(n)]


def _build(KC, with_bias=True):
    import contextlib

    import concourse.bacc as bacc
    import concourse.bass as bass
    import concourse.tile as tile
    import concourse.mybir as mybir

    F32 = mybir.dt.float32
    BF16 = mybir.dt.bfloat16
    AF = mybir.ActivationFunctionType
    ts = bass.ts

    KCT = KC // 128
    kch = _chunks(KC)

    nc = bacc.Bacc("TRN2", target_bir_lowering=False)

    xq_d = nc.dram_tensor("xq", [D, QW], BF16, kind="ExternalInput")
    xkv_d = nc.dram_tensor("xkv", [D, KC], BF16, kind="ExternalInput")
    wq_d = nc.dram_tensor("wq", [D, 512], BF16, kind="ExternalInput")
    wk_d = nc.dram_tensor("wk", [D, 512], BF16, kind="ExternalInput")
    wv_d = nc.dram_tensor("wv", [D, 512], BF16, kind="ExternalInput")
    wf_d = nc.dram_tensor("wf", [512, D], BF16, kind="ExternalInput")
    mb_d = nc.dram_tensor("mb", [128, KCT], F32, kind="ExternalInput")
    ones_d = nc.dram_tensor("ones", [128, 128], BF16, kind="ExternalInput")
    bq_d = nc.dram_tensor("bqt", [128, MT], F32, kind="ExternalInput")
    bk_d = nc.dram_tensor("bkt", [128, MT], F32, kind="ExternalInput")
    bv_d = nc.dram_tensor("bvr", [1, 512], BF16, kind="ExternalInput")
    out_d = nc.dram_tensor("out", [QW, D], BF16, kind="ExternalOutput")

    with tile.TileContext(nc) as tc, nc.allow_low_precision(
        reason="bf16 matmul pipeline with fp32 accumulation"
    ), contextlib.ExitStack() as ctx:
        const = ctx.enter_context(tc.tile_pool(name="const", bufs=1))
        persist = ctx.enter_context(tc.tile_pool(name="persist", bufs=1))
        ppool = ctx.enter_context(tc.tile_pool(name="ppool", bufs=4))
        rpool = ctx.enter_context(tc.tile_pool(name="rpool", bufs=3))
        bcpool = ctx.enter_context(tc.tile_pool(name="bcpool", bufs=4))
        outpool = ctx.enter_context(tc.tile_pool(name="outpool", bufs=2))
        psum = ctx.enter_context(tc.tile_pool(name="psum", bufs=2, space="PSUM"))
        drpool = ctx.enter_context(tc.tile_pool(name="drpool", bufs=3,
                                                space="DRAM"))

        q_t = [persist.tile([128, QW], BF16, name=f"q_t{m}", tag=f"q_t{m}")
               for m in range(MT)]
        k_t = [persist.tile([128, KC], BF16, name=f"k_t{m}", tag=f"k_t{m}")
               for m in range(MT)]
        v65 = [persist.tile([128, HL * 65], BF16, name=f"v65_{t}",
                            tag=f"v65_{t}") for t in range(KCT)]
        o_t = [persist.tile([128, QW], BF16, name=f"o_t{p}", tag=f"o_t{p}")
               for p in range(NP)]
        xkv = [persist.tile([128, KC], BF16, name=f"xkv{d}", tag=f"xkv{d}")
               for d in range(DT)]
        xq = [persist.tile([128, QW], BF16, name=f"xq{d}", tag=f"xq{d}")
              for d in range(DT)]
        wv = [persist.tile([128, 512], BF16, name=f"wv{d}", tag=f"wv{d}")
              for d in range(DT)]
        wk = [persist.tile([128, 512], BF16, name=f"wk{d}", tag=f"wk{d}")
              for d in range(DT)]
        wq = [persist.tile([128, 512], BF16, name=f"wq{d}", tag=f"wq{d}")
              for d in range(DT)]
        wf = [persist.tile([128, D], BF16, name=f"wf{d}", tag=f"wf{d}")
              for d in range(MT)]

        # a3 deps stream first: one full-KC DMA per xkv d-tile + wv
        for d in range(DT):
            nc.sync.dma_start(xkv[d][:], xkv_d[ts(d, 128), :])
            nc.sync.dma_start(wv[d][:], wv_d[ts(d, 128), :])
        # consts off the critical sync queue
        mb_sb = const.tile([128, KCT], F32, name="mb_sb")
        nc.scalar.dma_start(mb_sb[:], mb_d[:])
        ones128 = const.tile([128, 128], BF16, name="ones128")
        bq_sb = const.tile([128, MT], F32, name="bq_sb")
        bk_sb = const.tile([128, MT], F32, name="bk_sb")
        bv_sb = const.tile([1, 512], BF16, name="bv_sb")
        if with_bias:
            nc.scalar.dma_start(ones128[:], ones_d[:])
            nc.scalar.dma_start(bq_sb[:], bq_d[:])
            nc.scalar.dma_start(bk_sb[:], bk_d[:])
            nc.scalar.dma_start(bv_sb[:], bv_d[:])
        for t in range(KCT):
            v3 = v65[t].rearrange("p (h e) -> p h e", e=65)
            nc.vector.memset(v3[:, :, 64:65], 1.0)

        # ---------------- phase pieces ----------------
        def a3(trange):
            for t in trange:
                pv = psum.tile([128, 512], F32, name=f"pv{t}", tag="ps1024")
                for d in range(DT):
                    nc.tensor.matmul(pv[:], xkv[d][:, ts(t, 128)], wv[d][:],
                                     start=(d == 0),
                                     stop=(not with_bias and d == DT - 1))
                if with_bias:
                    nc.tensor.matmul(pv[:], ones128[0:1, 0:128], bv_sb[0:1, :],
                                     start=False, stop=True)
                dst = v65[t].rearrange("p (h e) -> p h e", e=65)
                src = pv.rearrange("p (h e) -> p h e", e=64)
                nc.scalar.copy(dst[:, :, 0:64], src[:])

        def a2(mrange, scalar_evict=False):
            for m in mrange:
                pk = [psum.tile([128, w], F32, name=f"pk{m}_{c}", tag="ps1024")
                      for c, w in enumerate(kch)]
                for d in range(DT):
                    off = 0
                    for c, w in enumerate(kch):
                        nc.tensor.matmul(
                            pk[c][:], wk[d][:, ts(m, 128)],
                            xkv[d][:, off:off + w],
                            start=(d == 0), stop=(d == DT - 1))
                        off += w
                off = 0
                for c, w in enumerate(kch):
                    dst = k_t[m][:, off:off + w]
                    if with_bias:
                        nc.vector.tensor_scalar_add(dst, pk[c][:],
                                                    bk_sb[:, m:m + 1])
                    elif scalar_evict:
                        nc.scalar.copy(dst, pk[c][:])
                    else:
                        nc.vector.tensor_copy(dst, pk[c][:])
                    off += w

        def a1_half(m, h2, scalar_evict=False, tag="ps1024"):
            # generator: one [128, 1024] query-column half of q_t[m];
            # yields after every matmul so it can hide in B-phase slack
            pq = psum.tile([128, 1024], F32, name=f"pq{m}_{h2}", tag=tag,
                           bufs=2)
            for d in range(DT):
                for c in range(2):
                    nc.tensor.matmul(
                        pq[:, ts(c, 512)], wq[d][:, ts(m, 128)],
                        xq[d][:, h2 * 1024 + c * 512:
                              h2 * 1024 + (c + 1) * 512],
                        start=(d == 0), stop=(d == DT - 1))
                    yield
            dst = q_t[m][:, h2 * 1024:(h2 + 1) * 1024]
            if with_bias:
                nc.vector.tensor_scalar_add(dst, pq[:], bq_sb[:, m:m + 1])
            elif scalar_evict:
                nc.scalar.copy(dst, pq[:])
            else:
                nc.vector.tensor_copy(dst, pq[:])
            yield

        def c_tile(qt, scalar_evict=False, tag="ps1024"):
            # generator: one [128 q, 1024] FC output tile
            fp = psum.tile([128, 1024], F32, name=f"fp{qt}", tag=tag, bufs=2)
            for dt in range(MT):
                for c in range(2):
                    nc.tensor.matmul(fp[:, ts(c, 512)],
                                     o_t[dt][:, ts(qt, 128)],
                                     wf[dt][:, ts(c, 512)],
                                     start=(dt == 0), stop=(dt == MT - 1))
                    yield
            ost = outpool.tile([128, 1024], BF16, name=f"ost{qt}", tag="ost")
            if scalar_evict:
                nc.scalar.copy(ost[:], fp[:])
            else:
                nc.vector.tensor_copy(ost[:], fp[:])
            nc.sync.dma_start(out_d[ts(qt, 128), :], ost[:])
            yield

        def run_all(gen):
            for _ in gen:
                pass

        pending_norm = []

        def b_group(qc, p, filler=None, fill_per_step=1):
            # one head pair x KCT key tiles; PV lags S/exp by one tile
            oacc = psum.tile([128, 1024], F32, name=f"o2_{qc}_{p}",
                             tag="oB", bufs=2)
            h0, h1 = 2 * p, 2 * p + 1

            def emit_pv(kt, pp):
                nc.tensor.matmul(
                    oacc[0:65, 0:512], v65[kt][:, h0 * 65:(h0 + 1) * 65],
                    pp[:, 0:512],
                    start=(kt == 0), stop=(kt == KCT - 1))
                nc.tensor.matmul(
                    oacc[0:65, 512:1024], v65[kt][:, h1 * 65:(h1 + 1) * 65],
                    pp[:, 512:1024],
                    start=(kt == 0), stop=(kt == KCT - 1))

            # PV lags the S/exp stream by LAG key tiles: keeps ScalarE fed
            # and gives the previous group's accumulator time to evict
            LAG = 2
            pps = {}
            for kt in range(KCT):
                s = psum.tile([128, 1024], F32, name=f"s_{qc}_{p}_{kt}",
                              tag="ps1024")
                nc.tensor.matmul(s[:, 0:512], k_t[p][0:64, ts(kt, 128)],
                                 q_t[p][0:64, ts(qc, 512)])
                nc.tensor.matmul(s[:, 512:1024], k_t[p][64:128, ts(kt, 128)],
                                 q_t[p][64:128, ts(qc, 512)])
                pp = ppool.tile([128, 1024], BF16, name=f"pp_{qc}_{p}_{kt}",
                                tag="pp")
                nc.scalar.activation(pp[:], s[:], AF.Exp,
                                     bias=mb_sb[:, kt:kt + 1],
                                     scale=float(SCALE))
                pps[kt] = pp
                if filler is not None:
                    for _ in range(fill_per_step):
                        try:
                            next(filler)
                        except StopIteration:
                            filler = None
                            break
                if kt >= LAG:
                    emit_pv(kt - LAG, pps.pop(kt - LAG))
            for kt in range(max(0, KCT - LAG), KCT):
                emit_pv(kt, pps.pop(kt))

            # stage denominator row + evacuate O^T (both heads)
            den = rpool.tile([1, 1024], F32, name=f"den_{qc}_{p}", tag="den",
                             bufs=3)
            nc.vector.tensor_copy(den[:], oacc[64:65, :])
            qsl = ts(qc, 512)
            nc.vector.tensor_copy(o_t[p][0:64, qsl], oacc[0:64, 0:512])
            nc.vector.tensor_copy(o_t[p][64:128, qsl], oacc[0:64, 512:1024])
            pending_norm.append((qc, p, den))
            return filler

        def b_norm_late():
            # reciprocal on DVE; broadcast via DRAM bounce; normalization
            # multiplies on the otherwise-idle GpSimd
            while pending_norm:
                qc, p, den = pending_norm.pop(0)
                rec = rpool.tile([1, 1024], F32, name=f"rec_{qc}_{p}",
                                 tag="rec")
                nc.vector.reciprocal_approx_fast(rec[:], den[:])
                rdr = drpool.tile([1, 1024], F32, name=f"rdr_{qc}_{p}",
                                  tag="rdr")
                nc.sync.dma_start(rdr[:], rec[:])
                qsl = ts(qc, 512)
                bcs = bcpool.tile([128, 512], F32, name=f"bcs_{qc}_{p}",
                                  tag="bcs")
                nc.sync.dma_start(bcs[0:64, :],
                                  rdr[0:1, 0:512].to_broadcast([64, 512]))
                nc.sync.dma_start(bcs[64:128, :],
                                  rdr[0:1, 512:1024].to_broadcast([64, 512]))
                nc.gpsimd.tensor_mul(o_t[p][0:64, qsl],
                                     o_t[p][0:64, qsl], bcs[0:64, :])
                nc.gpsimd.tensor_mul(o_t[p][64:128, qsl],
                                     o_t[p][64:128, qsl], bcs[64:128, :])

        def chain(*gens):
            for g in gens:
                yield from g

        # ---------------- emission order ----------------
        a3(range(KCT))
        for d in range(DT):
            nc.sync.dma_start(wk[d][:], wk_d[ts(d, 128), :])
        a2(range(0, 2), scalar_evict=True)
        for d in range(DT):
            nc.sync.dma_start(xq[d][:], xq_d[ts(d, 128), :])
            nc.sync.dma_start(wq[d][:], wq_d[ts(d, 128), :])
        a2(range(2, 4), scalar_evict=True)
        for m in range(MT):
            run_all(a1_half(m, 0, scalar_evict=True))
        for m in range(MT):
            nc.sync.dma_start(wf[m][:], wf_d[ts(m, 128), :])

        # B: 16 groups (qc x pair). Fillers ride the exp-wall slack through
        # the spare oB slot. a1 h1 must complete before qc2 (group 9);
        # c tiles for qc K become legal after b_norm_late of qc K.
        fillers = {
            (0, 0): a1_half(0, 1, tag="oB"),
            (0, 2): a1_half(1, 1, tag="oB"),
            (1, 0): a1_half(2, 1, tag="oB"),
            (1, 2): a1_half(3, 1, tag="oB"),
            (2, 0): chain(c_tile(0, tag="oB"), c_tile(1, tag="oB")),
            (2, 2): chain(c_tile(2, tag="oB"), c_tile(3, tag="oB")),
            (3, 0): chain(c_tile(4, tag="oB"), c_tile(5, tag="oB")),
            (3, 2): chain(c_tile(6, tag="oB"), c_tile(7, tag="oB")),
        }
        cur = None
        for qc in range(4):
            for p in range(NP):
                if (qc, p) in fillers:
                    cur = fillers[(qc, p)]
                cur = b_group(qc, p, filler=cur, fill_per_step=2)
                if qc == 3:
                    # last qc: stream each pair's norm immediately so only
                    # the final pair's chain is exposed past the last group
                    b_norm_late()
            b_norm_late()
        for qt in range(8, 16):
            run_all(c_tile(qt, scalar_evict=True))

    nc.finalize()
    return nc


_LDW_PATCHED = False


def _enable_ldw_opt():
    global _LDW_PATCHED
    if _LDW_PATCHED:
        return
    import concourse.bass_utils as bu
    orig = bu.run_command

    def patched(cmd, *a, **k):
        cmd = [c.replace("--enable-ldw-opt=false", "--enable-ldw-opt=true")
               if isinstance(c, str) else c for c in cmd]
        return orig(cmd, *a, **k)

    bu.run_command = patched
    _LDW_PATCHED = True


def _get_program(KC, with_bias=True):
    key = (KC, with_bias)
    if key not in _programs:
        if __import__("os").environ.get("LDW_OPT"):
            _enable_ldw_opt()
        _programs[key] = _build(KC, with_bias)
    return _programs[key]


LAST_EXEC_NS = None
PROFILE = False


def _ensure_profile_hook():
    import types

    try:
        from antenv.axon_hooks import get_axon_ntff_profile_hook  # noqa: F401
        return
    except ImportError:
        pass
    import antenv

    mod = types.ModuleType("antenv.axon_hooks")
    _h = [None]
    mod.set_axon_ntff_profile_hook = lambda h: _h.__setitem__(0, h)
    mod.get_axon_ntff_profile_hook = lambda: _h[0]
    sys.modules["antenv.axon_hooks"] = mod
    antenv.axon_hooks = mod
    from trn_agent_boot.trn_boot import _ntff_profile_via_ctypes

    mod.set_axon_ntff_profile_hook(
        _ntff_profile_via_ctypes("/opt/axon/libaxon_pjrt.so"))
    import concourse.bass_utils as bu

    bu.upload_artifacts = lambda tmpdir: f"local:{tmpdir}"


def kernel(x, mask, Wq, bq, Wk, bk, Wv, bv, Wf, bf):
    global LAST_EXEC_NS
    from concourse.bass_utils import run_bass_kernel_spmd

    if PROFILE:
        _ensure_profile_hook()

    x = np.asarray(x, dtype=np.float32)
    mask = np.asarray(mask)
    Wq16, Wk16, Wv16, Wf16 = (
        np.ascontiguousarray(np.asarray(w).astype(BF16NP))
        for w in (Wq, Wk, Wv, Wf))
    bq32, bk32 = (np.asarray(v, np.float32) for v in (bq, bk))
    bv32 = np.asarray(bv, np.float32)
    bf32 = np.asarray(bf, np.float32)

    keeps = [np.nonzero(np.asarray(mask[b]) == 0)[0] for b in range(BS)]
    maxk = max(1, max(len(k) for k in keeps))
    KC = -(-maxk // 128) * 128
    with_bias = bool(np.any(bq32) or np.any(bk32) or np.any(bv32))
    nc = _get_program(KC, with_bias)
    KCT = KC // 128

    x16 = x.astype(BF16NP)
    in_maps = []
    for c in range(8):
        b, j = divmod(c, 2)
        keep = keeps[b]
        xq_t = np.ascontiguousarray(x16[b].T)
        xkv_t = np.zeros((D, KC), BF16NP)
        xkv_t[:, :len(keep)] = x16[b, keep, :].T
        mbv = np.full(KC, NEG, np.float32)
        mbv[:len(keep)] = 0.0
        mb_t = np.ascontiguousarray(mbv.reshape(KCT, 128).T)
        sl = slice(512 * j, 512 * (j + 1))
        in_maps.append({
            "ones": _ONES, "xq": xq_t, "xkv": xkv_t,
            "wq": np.ascontiguousarray(Wq16[:, sl]),
            "wk": np.ascontiguousarray(Wk16[:, sl]),
            "wv": np.ascontiguousarray(Wv16[:, sl]),
            "wf": np.ascontiguousarray(Wf16[sl, :]),
            "mb": mb_t,
            "bqt": np.ascontiguousarray(bq32[sl].reshape(MT, 128).T),
            "bkt": np.ascontiguousarray(bk32[sl].reshape(MT, 128).T),
            "bvr": bv32[sl].astype(BF16NP).reshape(1, 512),
        })

    res = run_bass_kernel_spmd(nc, in_maps, core_ids=list(range(8)),
                               trace=PROFILE)
    if res.exec_time_ns is not None:
        LAST_EXEC_NS = res.exec_time_ns

    out = np.empty((BS, L, D), np.float32)
    for b in range(BS):
        out[b] = (res.results[2 * b]["out"].astype(np.float32)
                  + res.results[2 * b + 1]["out"].astype(np.float32))
    out += bf32.reshape(1, 1, D)
    return out


# revision 4
# speedup vs baseline: 1.0227x; 1.0087x over previous
"""Trainium2 Bass kernel for nn_MultiHeadAttention (BS=4, L=2048, D=1024, H=16).

Sharding: 8 cores = 4 batches x 2 head-halves. Core (b, j) computes heads
8j..8j+8 of batch b for ALL 2048 queries, K/V over the batch's unmasked keys
(host gather). Wq/Wk/Wv column-sharded, Wf row-sharded; each core emits a
bf16 PARTIAL out = O_local @ Wf_rows and the host adds the two partials per
batch (+ bf).

Schedule: a3 (V), a2 (K), a1-half (Q cols 0:1024) run upfront, PE-dense.
The B phase processes one head-pair per group (16 groups of 9 key tiles);
it is exp-wall-bound on ScalarE (one ACTIVATE per key tile). PV matmuls lag
the S/exp stream by one key tile (software pipelining) so ScalarE never
starves, and the PE slack under the exp wall is filled by injecting the
remaining a1 matmuls and the first 8 FC (C) tiles one matmul per step
through the spare oB PSUM slot (one pair per group leaves one of the two oB
slots free to rotate through filler tiles). Denominator staging is a small
Vector copy; o_t normalization multiplies run on the otherwise-idle GpSimd
so the Vector FIFO cannot block PSUM evictions.
"""
import sys

sys.path.insert(0, "/opt/trn_rl_repo")

import numpy as np
import ml_dtypes

BF16NP = ml_dtypes.bfloat16

BS, L, D, H, DK = 4, 2048, 1024, 16, 64
SCALE = 1.0 / np.sqrt(DK)
QW = L             # queries per core
HL = 8             # local heads
NP = HL // 2       # local head pairs
MT = 4             # local out-dim tiles (512/128)
DT = D // 128      # contraction d tiles
NEG = -1.0e30

_programs = {}
_ONES = np.ones((128, 128), BF16NP)


def _chunks(total, maxc=512):
    n = -(-total // maxc)
    base = total // n
    rem = total - base * n
    return [base + (1 if i < rem else 0) for i in range(n)]




# ---- custom-DVE exp2: bf16-bits construction in one 8-stage DVE op ----
# value(Z-bits) = KAPPA * 2^(y/128) with |rel err| <= 0.0054 (incl. bf16
# rounding) for |y| <= 1100. The K projection is host-prescaled by ALPHA so
# the PSUM logit arrives as y = 128*log2(e)*SCALE*s_raw; ScalarE kts match
# via exp(y*ln2/128 + ln KAPPA).
EXP2_MAGIC = 1.5 * 2**30 + 16128.0
EXP2_H = 186.0380113
EXP2_A = 0.0026865429
EXP2_C = 16088.5532310
EXP2_KAPPA = 0.7088638733
ALPHA = 128.0 * np.log2(np.e) * (1.0 / np.sqrt(DK))
LN2_128 = np.log(2.0) / 128.0
LNKAPPA = float(np.log(EXP2_KAPPA))

_EXP2_OP = None


def _install_exp2():
    global _EXP2_OP
    if _EXP2_OP is not None:
        return _EXP2_OP
    import concourse.dve_ops as dops
    import concourse.dve_spec as dspec
    from concourse.dve_spec import Spec, Src0, Src1, C0, C1, C2
    from concourse.dve_uop import DveOpSpec

    name = "EXP2_BITS_ANT"
    T = Src0 + Src1
    N = T - Src1
    R = Src0 - N
    u = R + C1
    Z = N + ((u * u) * C2 + C0)

    def ref(in0, in1, c0, c1, c2):
        S = np.asarray(in0, np.float32)
        M = np.asarray(in1, np.float32)
        Tv = (S + M).astype(np.float32)
        Nv = (Tv - M).astype(np.float32)
        Rv = (S - Nv).astype(np.float32)
        uv = (Rv + np.float32(c1)).astype(np.float32)
        Zv = (Nv + (uv * uv * np.float32(c2) + np.float32(c0))).astype(
            np.float32)
        return np.clip(np.rint(Zv), 0, 65535)

    spec = Spec(body=Z, reference=ref)
    if name not in dops._SUB_OPCODE_FOR_NAME:
        row = max(dops._SUB_OPCODE_FOR_NAME.values()) + 1
        assert row < 0x20
        dops._SUB_OPCODE_FOR_NAME[name] = row
    row = dops._SUB_OPCODE_FOR_NAME[name]
    from concourse.dve_table_gen import dve_ver_for as _vf
    try:
        ver = _vf("TRN2")
    except Exception:
        ver = "v3"
    uops = dspec.lower(spec, ver=ver)
    sha = DveOpSpec(name=name, opcode=row, uops=uops,
                    rd1_en=dspec._has_src1(spec)).sha(ver)
    op = dops.DveOp(name, spec, subdim=False, uops_sha={ver: sha})
    if all(o.name != name for o in dops.OPS):
        dops.OPS.append(op)
    dops.CUSTOM_DVE_SPECS[name] = spec
    _EXP2_OP = op
    return op


def _build(KC, with_bias=True):
    import contextlib

    import concourse.bacc as bacc
    import concourse.bass as bass
    import concourse.tile as tile
    import concourse.mybir as mybir

    F32 = mybir.dt.float32
    BF16 = mybir.dt.bfloat16
    AF = mybir.ActivationFunctionType
    ts = bass.ts

    KCT = KC // 128
    kch = _chunks(KC)

    nc = bacc.Bacc("TRN2", target_bir_lowering=False)

    xq_d = nc.dram_tensor("xq", [D, QW], BF16, kind="ExternalInput")
    xkv_d = nc.dram_tensor("xkv", [D, KC], BF16, kind="ExternalInput")
    wq_d = nc.dram_tensor("wq", [D, 512], BF16, kind="ExternalInput")
    wk_d = nc.dram_tensor("wk", [D, 512], BF16, kind="ExternalInput")
    wv_d = nc.dram_tensor("wv", [D, 512], BF16, kind="ExternalInput")
    wf_d = nc.dram_tensor("wf", [512, D], BF16, kind="ExternalInput")
    mb_d = nc.dram_tensor("mb", [128, KCT], F32, kind="ExternalInput")
    ones_d = nc.dram_tensor("ones", [128, 128], BF16, kind="ExternalInput")
    bq_d = nc.dram_tensor("bqt", [128, MT], F32, kind="ExternalInput")
    bk_d = nc.dram_tensor("bkt", [128, MT], F32, kind="ExternalInput")
    bv_d = nc.dram_tensor("bvr", [1, 512], BF16, kind="ExternalInput")
    npd_d = nc.dram_tensor("npd", [1, 1], F32, kind="ExternalInput")
    out_d = nc.dram_tensor("out", [QW, D], BF16, kind="ExternalOutput")

    with tile.TileContext(nc) as tc, nc.allow_low_precision(
        reason="bf16 matmul pipeline with fp32 accumulation"
    ), contextlib.ExitStack() as ctx:
        const = ctx.enter_context(tc.tile_pool(name="const", bufs=1))
        persist = ctx.enter_context(tc.tile_pool(name="persist", bufs=1))
        ppool = ctx.enter_context(tc.tile_pool(name="ppool", bufs=4))
        rpool = ctx.enter_context(tc.tile_pool(name="rpool", bufs=3))
        bcpool = ctx.enter_context(tc.tile_pool(name="bcpool", bufs=4))
        outpool = ctx.enter_context(tc.tile_pool(name="outpool", bufs=2))
        psum = ctx.enter_context(tc.tile_pool(name="psum", bufs=2, space="PSUM"))
        drpool = ctx.enter_context(tc.tile_pool(name="drpool", bufs=3,
                                                space="DRAM"))

        q_t = [persist.tile([128, QW], BF16, name=f"q_t{m}", tag=f"q_t{m}")
               for m in range(MT)]
        k_t = [persist.tile([128, KC], BF16, name=f"k_t{m}", tag=f"k_t{m}")
               for m in range(MT)]
        v65 = [persist.tile([128, HL * 65], BF16, name=f"v65_{t}",
                            tag=f"v65_{t}") for t in range(KCT)]
        o_t = [persist.tile([128, QW], BF16, name=f"o_t{p}", tag=f"o_t{p}")
               for p in range(NP)]
        xkv = [persist.tile([128, KC], BF16, name=f"xkv{d}", tag=f"xkv{d}")
               for d in range(DT)]
        xq = [persist.tile([128, QW], BF16, name=f"xq{d}", tag=f"xq{d}")
              for d in range(DT)]
        wv = [persist.tile([128, 512], BF16, name=f"wv{d}", tag=f"wv{d}")
              for d in range(DT)]
        wk = [persist.tile([128, 512], BF16, name=f"wk{d}", tag=f"wk{d}")
              for d in range(DT)]
        wq = [persist.tile([128, 512], BF16, name=f"wq{d}", tag=f"wq{d}")
              for d in range(DT)]
        wf = [persist.tile([128, D], BF16, name=f"wf{d}", tag=f"wf{d}")
              for d in range(MT)]

        # a3 deps stream first: one full-KC DMA per xkv d-tile + wv
        for d in range(DT):
            nc.sync.dma_start(xkv[d][:], xkv_d[ts(d, 128), :])
            nc.sync.dma_start(wv[d][:], wv_d[ts(d, 128), :])
        # consts off the critical sync queue
        mb_sb = const.tile([128, KCT], F32, name="mb_sb")
        npd_sb = const.tile([1, 1], F32, name="npd_sb")
        magic_sb = const.tile([128, 1024], F32, name="magic_sb")
        lnk_sb = const.tile([128, 1], F32, name="lnk_sb")
        if with_bias:
            nc.scalar.dma_start(mb_sb[:], mb_d[:])
        else:
            nc.scalar.dma_start(npd_sb[:], npd_d[:])
            nc.vector.memset(magic_sb[:], float(EXP2_MAGIC))
            nc.vector.memset(lnk_sb[:], float(LNKAPPA))
        exp2op = None if with_bias else _install_exp2()
        U16 = mybir.dt.uint16
        ones128 = const.tile([128, 128], BF16, name="ones128")
        bq_sb = const.tile([128, MT], F32, name="bq_sb")
        bk_sb = const.tile([128, MT], F32, name="bk_sb")
        bv_sb = const.tile([1, 512], BF16, name="bv_sb")
        if with_bias:
            nc.scalar.dma_start(ones128[:], ones_d[:])
            nc.scalar.dma_start(bq_sb[:], bq_d[:])
            nc.scalar.dma_start(bk_sb[:], bk_d[:])
            nc.scalar.dma_start(bv_sb[:], bv_d[:])
        for t in range(KCT):
            v3 = v65[t].rearrange("p (h e) -> p h e", e=65)
            nc.vector.memset(v3[:, :, 64:65], 1.0)

        # ---------------- phase pieces ----------------
        def a3(trange):
            for t in trange:
                pv = psum.tile([128, 512], F32, name=f"pv{t}", tag="ps1024")
                for d in range(DT):
                    nc.tensor.matmul(pv[:], xkv[d][:, ts(t, 128)], wv[d][:],
                                     start=(d == 0),
                                     stop=(not with_bias and d == DT - 1))
                if with_bias:
                    nc.tensor.matmul(pv[:], ones128[0:1, 0:128], bv_sb[0:1, :],
                                     start=False, stop=True)
                dst = v65[t].rearrange("p (h e) -> p h e", e=65)
                src = pv.rearrange("p (h e) -> p h e", e=64)
                nc.scalar.copy(dst[:, :, 0:64], src[:])

        def a2(mrange, scalar_evict=False):
            for m in mrange:
                pk = [psum.tile([128, w], F32, name=f"pk{m}_{c}", tag="ps1024")
                      for c, w in enumerate(kch)]
                for d in range(DT):
                    off = 0
                    for c, w in enumerate(kch):
                        nc.tensor.matmul(
                            pk[c][:], wk[d][:, ts(m, 128)],
                            xkv[d][:, off:off + w],
                            start=(d == 0), stop=(d == DT - 1))
                        off += w
                off = 0
                for c, w in enumerate(kch):
                    dst = k_t[m][:, off:off + w]
                    if with_bias:
                        nc.vector.tensor_scalar_add(dst, pk[c][:],
                                                    bk_sb[:, m:m + 1])
                    elif scalar_evict:
                        nc.scalar.copy(dst, pk[c][:])
                    else:
                        nc.vector.tensor_copy(dst, pk[c][:])
                    off += w

        def a1_half(m, h2, scalar_evict=False, tag="ps1024"):
            # generator: one [128, 1024] query-column half of q_t[m];
            # yields after every matmul so it can hide in B-phase slack
            pq = psum.tile([128, 1024], F32, name=f"pq{m}_{h2}", tag=tag,
                           bufs=2)
            for d in range(DT):
                for c in range(2):
                    nc.tensor.matmul(
                        pq[:, ts(c, 512)], wq[d][:, ts(m, 128)],
                        xq[d][:, h2 * 1024 + c * 512:
                              h2 * 1024 + (c + 1) * 512],
                        start=(d == 0), stop=(d == DT - 1))
                    yield
            dst = q_t[m][:, h2 * 1024:(h2 + 1) * 1024]
            if with_bias:
                nc.vector.tensor_scalar_add(dst, pq[:], bq_sb[:, m:m + 1])
            elif scalar_evict:
                nc.scalar.copy(dst, pq[:])
            else:
                nc.vector.tensor_copy(dst, pq[:])
            yield

        def c_tile(qt, scalar_evict=False, tag="ps1024"):
            # generator: one [128 q, 1024] FC output tile
            fp = psum.tile([128, 1024], F32, name=f"fp{qt}", tag=tag, bufs=2)
            for dt in range(MT):
                for c in range(2):
                    nc.tensor.matmul(fp[:, ts(c, 512)],
                                     o_t[dt][:, ts(qt, 128)],
                                     wf[dt][:, ts(c, 512)],
                                     start=(dt == 0), stop=(dt == MT - 1))
                    yield
            ost = outpool.tile([128, 1024], BF16, name=f"ost{qt}", tag="ost")
            if scalar_evict:
                nc.scalar.copy(ost[:], fp[:])
            else:
                nc.vector.tensor_copy(ost[:], fp[:])
            nc.sync.dma_start(out_d[ts(qt, 128), :], ost[:])
            yield

        def run_all(gen):
            for _ in gen:
                pass

        pending_norm = []

        def b_group(qc, p, filler=None, fill_per_step=1):
            # one head pair x KCT key tiles; PV lags S/exp by one tile
            oacc = psum.tile([128, 1024], F32, name=f"o2_{qc}_{p}",
                             tag="oB", bufs=2)
            h0, h1 = 2 * p, 2 * p + 1

            def emit_pv(kt, pp):
                nc.tensor.matmul(
                    oacc[0:65, 0:512], v65[kt][:, h0 * 65:(h0 + 1) * 65],
                    pp[:, 0:512],
                    start=(kt == 0), stop=(kt == KCT - 1))
                nc.tensor.matmul(
                    oacc[0:65, 512:1024], v65[kt][:, h1 * 65:(h1 + 1) * 65],
                    pp[:, 512:1024],
                    start=(kt == 0), stop=(kt == KCT - 1))

            # PV lags the S/exp stream by LAG key tiles: keeps ScalarE fed
            # and gives the previous group's accumulator time to evict
            LAG = 2
            pps = {}
            for kt in range(KCT):
                s = psum.tile([128, 1024], F32, name=f"s_{qc}_{p}_{kt}",
                              tag="ps1024")
                nc.tensor.matmul(s[:, 0:512], k_t[p][0:64, ts(kt, 128)],
                                 q_t[p][0:64, ts(qc, 512)])
                nc.tensor.matmul(s[:, 512:1024], k_t[p][64:128, ts(kt, 128)],
                                 q_t[p][64:128, ts(qc, 512)])
                pp = ppool.tile([128, 1024], BF16, name=f"pp_{qc}_{p}_{kt}",
                                tag="pp")
                if with_bias:
                    nc.scalar.activation(pp[:], s[:], AF.Exp,
                                         bias=mb_sb[:, kt:kt + 1],
                                         scale=float(SCALE))
                elif kt % 3 == 2:
                    nc.vector._custom_dve(
                        exp2op, out=pp.bitcast(U16), in0=s[:],
                        in1=magic_sb[:], s0=float(EXP2_C),
                        s1=float(EXP2_H), imm2=float(EXP2_A))
                else:
                    nc.scalar.activation(pp[:], s[:], AF.Exp,
                                         bias=lnk_sb[:, 0:1],
                                         scale=float(LN2_128))
                pps[kt] = pp
                if filler is not None:
                    for _ in range(fill_per_step):
                        try:
                            next(filler)
                        except StopIteration:
                            filler = None
                            break
                if kt >= LAG:
                    emit_pv(kt - LAG, pps.pop(kt - LAG))
            for kt in range(max(0, KCT - LAG), KCT):
                emit_pv(kt, pps.pop(kt))

            # stage denominator row + evacuate O^T (both heads)
            den = rpool.tile([1, 1024], F32, name=f"den_{qc}_{p}", tag="den",
                             bufs=3)
            if with_bias:
                nc.vector.tensor_copy(den[:], oacc[64:65, :])
            else:
                nc.vector.tensor_scalar_add(den[:], oacc[64:65, :],
                                            npd_sb[0:1, 0:1])
            qsl = ts(qc, 512)
            nc.vector.tensor_copy(o_t[p][0:64, qsl], oacc[0:64, 0:512])
            nc.vector.tensor_copy(o_t[p][64:128, qsl], oacc[0:64, 512:1024])
            pending_norm.append((qc, p, den))
            return filler

        def b_norm_late():
            # reciprocal on DVE; broadcast via DRAM bounce; normalization
            # multiplies on the otherwise-idle GpSimd
            while pending_norm:
                qc, p, den = pending_norm.pop(0)
                rec = rpool.tile([1, 1024], F32, name=f"rec_{qc}_{p}",
                                 tag="rec")
                nc.vector.reciprocal_approx_fast(rec[:], den[:])
                rdr = drpool.tile([1, 1024], F32, name=f"rdr_{qc}_{p}",
                                  tag="rdr")
                nc.sync.dma_start(rdr[:], rec[:])
                qsl = ts(qc, 512)
                bcs = bcpool.tile([128, 512], F32, name=f"bcs_{qc}_{p}",
                                  tag="bcs")
                nc.sync.dma_start(bcs[0:64, :],
                                  rdr[0:1, 0:512].to_broadcast([64, 512]))
                nc.sync.dma_start(bcs[64:128, :],
                                  rdr[0:1, 512:1024].to_broadcast([64, 512]))
                nc.gpsimd.tensor_mul(o_t[p][0:64, qsl],
                                     o_t[p][0:64, qsl], bcs[0:64, :])
                nc.gpsimd.tensor_mul(o_t[p][64:128, qsl],
                                     o_t[p][64:128, qsl], bcs[64:128, :])

        def chain(*gens):
            for g in gens:
                yield from g

        # ---------------- emission order ----------------
        a3(range(KCT))
        for d in range(DT):
            nc.sync.dma_start(wk[d][:], wk_d[ts(d, 128), :])
        a2(range(0, 2), scalar_evict=True)
        for d in range(DT):
            nc.sync.dma_start(xq[d][:], xq_d[ts(d, 128), :])
            nc.sync.dma_start(wq[d][:], wq_d[ts(d, 128), :])
        a2(range(2, 4), scalar_evict=True)
        for m in range(MT):
            run_all(a1_half(m, 0, scalar_evict=True))
        for m in range(MT):
            nc.sync.dma_start(wf[m][:], wf_d[ts(m, 128), :])

        # B: 16 groups (qc x pair). Fillers ride the exp-wall slack through
        # the spare oB slot. a1 h1 must complete before qc2 (group 9);
        # c tiles for qc K become legal after b_norm_late of qc K.
        fillers = {
            (0, 0): a1_half(0, 1, tag="oB", scalar_evict=True),
            (0, 2): a1_half(1, 1, tag="oB", scalar_evict=True),
            (1, 0): a1_half(2, 1, tag="oB", scalar_evict=True),
            (1, 2): a1_half(3, 1, tag="oB", scalar_evict=True),
            (2, 0): chain(c_tile(0, tag="oB", scalar_evict=True),
                          c_tile(1, tag="oB", scalar_evict=True)),
            (2, 2): chain(c_tile(2, tag="oB", scalar_evict=True),
                          c_tile(3, tag="oB", scalar_evict=True)),
            (3, 0): chain(c_tile(4, tag="oB", scalar_evict=True),
                          c_tile(5, tag="oB", scalar_evict=True)),
            (3, 2): chain(c_tile(6, tag="oB", scalar_evict=True),
                          c_tile(7, tag="oB", scalar_evict=True)),
        }
        cur = None
        for qc in range(4):
            for p in range(NP):
                if (qc, p) in fillers:
                    cur = fillers[(qc, p)]
                cur = b_group(qc, p, filler=cur, fill_per_step=2)
                if qc == 3:
                    # last qc: stream each pair's norm immediately so only
                    # the final pair's chain is exposed past the last group
                    b_norm_late()
            b_norm_late()
        for qt in range(8, 16):
            run_all(c_tile(qt, scalar_evict=True))

    nc.finalize()
    return nc


_LDW_PATCHED = False


def _enable_ldw_opt():
    global _LDW_PATCHED
    if _LDW_PATCHED:
        return
    import concourse.bass_utils as bu
    orig = bu.run_command

    def patched(cmd, *a, **k):
        cmd = [c.replace("--enable-ldw-opt=false", "--enable-ldw-opt=true")
               if isinstance(c, str) else c for c in cmd]
        return orig(cmd, *a, **k)

    bu.run_command = patched
    _LDW_PATCHED = True


def _get_program(KC, with_bias=True):
    key = (KC, with_bias)
    if key not in _programs:
        if __import__("os").environ.get("LDW_OPT"):
            _enable_ldw_opt()
        _programs[key] = _build(KC, with_bias)
    return _programs[key]


LAST_EXEC_NS = None
PROFILE = False


def _ensure_profile_hook():
    import types

    try:
        from antenv.axon_hooks import get_axon_ntff_profile_hook  # noqa: F401
        return
    except ImportError:
        pass
    import antenv

    mod = types.ModuleType("antenv.axon_hooks")
    _h = [None]
    mod.set_axon_ntff_profile_hook = lambda h: _h.__setitem__(0, h)
    mod.get_axon_ntff_profile_hook = lambda: _h[0]
    sys.modules["antenv.axon_hooks"] = mod
    antenv.axon_hooks = mod
    from trn_agent_boot.trn_boot import _ntff_profile_via_ctypes

    mod.set_axon_ntff_profile_hook(
        _ntff_profile_via_ctypes("/opt/axon/libaxon_pjrt.so"))
    import concourse.bass_utils as bu

    bu.upload_artifacts = lambda tmpdir: f"local:{tmpdir}"


def kernel(x, mask, Wq, bq, Wk, bk, Wv, bv, Wf, bf):
    global LAST_EXEC_NS
    from concourse.bass_utils import run_bass_kernel_spmd

    if PROFILE:
        _ensure_profile_hook()

    x = np.asarray(x, dtype=np.float32)
    mask = np.asarray(mask)
    keeps0 = [np.nonzero(np.asarray(mask[b]) == 0)[0] for b in range(BS)]
    wb0 = bool(np.any(np.asarray(bq)) or np.any(np.asarray(bk))
               or np.any(np.asarray(bv)))
    wk_scale = 1.0 if wb0 else ALPHA
    Wq16, Wk16, Wv16, Wf16 = (
        np.ascontiguousarray(np.asarray(w, np.float32) * s_).astype(BF16NP)
        for w, s_ in ((Wq, 1.0), (Wk, wk_scale), (Wv, 1.0), (Wf, 1.0)))
    bq32, bk32 = (np.asarray(v, np.float32) for v in (bq, bk))
    bv32 = np.asarray(bv, np.float32)
    bf32 = np.asarray(bf, np.float32)

    keeps = [np.nonzero(np.asarray(mask[b]) == 0)[0] for b in range(BS)]
    maxk = max(1, max(len(k) for k in keeps))
    KC = -(-maxk // 128) * 128
    with_bias = bool(np.any(bq32) or np.any(bk32) or np.any(bv32))
    nc = _get_program(KC, with_bias)
    KCT = KC // 128

    x16 = x.astype(BF16NP)
    in_maps = []
    for c in range(8):
        b, j = divmod(c, 2)
        keep = keeps[b]
        xq_t = np.ascontiguousarray(x16[b].T)
        xkv_t = np.zeros((D, KC), BF16NP)
        xkv_t[:, :len(keep)] = x16[b, keep, :].T
        mbv = np.full(KC, NEG, np.float32)
        mbv[:len(keep)] = 0.0
        mb_t = np.ascontiguousarray(mbv.reshape(KCT, 128).T)
        sl = slice(512 * j, 512 * (j + 1))
        in_maps.append({
            "ones": _ONES, "xq": xq_t, "xkv": xkv_t,
            "npd": np.full((1, 1), -float(KC - len(keep))
                           * EXP2_KAPPA, np.float32),
            "wq": np.ascontiguousarray(Wq16[:, sl]),
            "wk": np.ascontiguousarray(Wk16[:, sl]),
            "wv": np.ascontiguousarray(Wv16[:, sl]),
            "wf": np.ascontiguousarray(Wf16[sl, :]),
            "mb": mb_t,
            "bqt": np.ascontiguousarray(bq32[sl].reshape(MT, 128).T),
            "bkt": np.ascontiguousarray(bk32[sl].reshape(MT, 128).T),
            "bvr": bv32[sl].astype(BF16NP).reshape(1, 512),
        })

    res = run_bass_kernel_spmd(nc, in_maps, core_ids=list(range(8)),
                               trace=PROFILE)
    if res.exec_time_ns is not None:
        LAST_EXEC_NS = res.exec_time_ns

    out = np.empty((BS, L, D), np.float32)
    for b in range(BS):
        out[b] = (res.results[2 * b]["out"].astype(np.float32)
                  + res.results[2 * b + 1]["out"].astype(np.float32))
    out += bf32.reshape(1, 1, D)
    return out


# revision 5
# speedup vs baseline: 1.0262x; 1.0034x over previous
"""Trainium2 Bass kernel for nn_MultiHeadAttention (BS=4, L=2048, D=1024, H=16).

Sharding: 8 cores = 4 batches x 2 head-halves. Core (b, j) computes heads
8j..8j+8 of batch b for ALL 2048 queries, K/V over the batch's unmasked keys
(host gather). Wq/Wk/Wv column-sharded, Wf row-sharded; each core emits a
bf16 PARTIAL out = O_local @ Wf_rows and the host adds the two partials per
batch (+ bf).

Schedule: a3 (V), a2 (K), a1-half (Q cols 0:1024) run upfront, PE-dense.
The B phase processes one head-pair per group (16 groups of 9 key tiles);
it is exp-wall-bound on ScalarE (one ACTIVATE per key tile). PV matmuls lag
the S/exp stream by one key tile (software pipelining) so ScalarE never
starves, and the PE slack under the exp wall is filled by injecting the
remaining a1 matmuls and the first 8 FC (C) tiles one matmul per step
through the spare oB PSUM slot (one pair per group leaves one of the two oB
slots free to rotate through filler tiles). Denominator staging is a small
Vector copy; o_t normalization multiplies run on the otherwise-idle GpSimd
so the Vector FIFO cannot block PSUM evictions.
"""
import sys

sys.path.insert(0, "/opt/trn_rl_repo")

import numpy as np
import ml_dtypes

BF16NP = ml_dtypes.bfloat16

BS, L, D, H, DK = 4, 2048, 1024, 16, 64
SCALE = 1.0 / np.sqrt(DK)
QW = L             # queries per core
HL = 8             # local heads
NP = HL // 2       # local head pairs
MT = 4             # local out-dim tiles (512/128)
DT = D // 128      # contraction d tiles
NEG = -1.0e30

_programs = {}
_ONES = np.ones((128, 128), BF16NP)


def _chunks(total, maxc=512):
    n = -(-total // maxc)
    base = total // n
    rem = total - base * n
    return [base + (1 if i < rem else 0) for i in range(n)]




# ---- custom-DVE exp2: bf16-bits construction in one 8-stage DVE op ----
# value(Z-bits) = KAPPA * 2^(y/128) with |rel err| <= 0.0054 (incl. bf16
# rounding) for |y| <= 1100. The K projection is host-prescaled by ALPHA so
# the PSUM logit arrives as y = 128*log2(e)*SCALE*s_raw; ScalarE kts match
# via exp(y*ln2/128 + ln KAPPA).
EXP2_MAGIC = 1.5 * 2**30 + 16128.0
EXP2_H = 186.0380113
EXP2_A = 0.0026865429
EXP2_C = 16088.5532310
EXP2_KAPPA = 0.7088638733
ALPHA = 128.0 * np.log2(np.e) * (1.0 / np.sqrt(DK))
LN2_128 = np.log(2.0) / 128.0
LNKAPPA = float(np.log(EXP2_KAPPA))

_EXP2_OP = None


def _install_exp2():
    global _EXP2_OP
    if _EXP2_OP is not None:
        return _EXP2_OP
    import concourse.dve_ops as dops
    import concourse.dve_spec as dspec
    from concourse.dve_spec import Spec, Src0, Src1, C0, C1, C2
    from concourse.dve_uop import DveOpSpec

    name = "EXP2_BITS_ANT"
    T = Src0 + Src1
    N = T - Src1
    R = Src0 - N
    u = R + C1
    Z = N + ((u * u) * C2 + C0)

    def ref(in0, in1, c0, c1, c2):
        S = np.asarray(in0, np.float32)
        M = np.asarray(in1, np.float32)
        Tv = (S + M).astype(np.float32)
        Nv = (Tv - M).astype(np.float32)
        Rv = (S - Nv).astype(np.float32)
        uv = (Rv + np.float32(c1)).astype(np.float32)
        Zv = (Nv + (uv * uv * np.float32(c2) + np.float32(c0))).astype(
            np.float32)
        return np.clip(np.rint(Zv), 0, 65535)

    spec = Spec(body=Z, reference=ref)
    if name not in dops._SUB_OPCODE_FOR_NAME:
        row = max(dops._SUB_OPCODE_FOR_NAME.values()) + 1
        assert row < 0x20
        dops._SUB_OPCODE_FOR_NAME[name] = row
    row = dops._SUB_OPCODE_FOR_NAME[name]
    from concourse.dve_table_gen import dve_ver_for as _vf
    try:
        ver = _vf("TRN2")
    except Exception:
        ver = "v3"
    uops = dspec.lower(spec, ver=ver)
    sha = DveOpSpec(name=name, opcode=row, uops=uops,
                    rd1_en=dspec._has_src1(spec)).sha(ver)
    op = dops.DveOp(name, spec, subdim=False, uops_sha={ver: sha})
    if all(o.name != name for o in dops.OPS):
        dops.OPS.append(op)
    dops.CUSTOM_DVE_SPECS[name] = spec
    _EXP2_OP = op
    return op


def _build(KC, with_bias=True):
    import contextlib

    import concourse.bacc as bacc
    import concourse.bass as bass
    import concourse.tile as tile
    import concourse.mybir as mybir

    F32 = mybir.dt.float32
    BF16 = mybir.dt.bfloat16
    AF = mybir.ActivationFunctionType
    ts = bass.ts

    KCT = KC // 128
    kch = _chunks(KC)

    nc = bacc.Bacc("TRN2", target_bir_lowering=False)

    xq_d = nc.dram_tensor("xq", [D, QW], BF16, kind="ExternalInput")
    xkv_d = nc.dram_tensor("xkv", [D, KC], BF16, kind="ExternalInput")
    wq_d = nc.dram_tensor("wq", [D, 512], BF16, kind="ExternalInput")
    wk_d = nc.dram_tensor("wk", [D, 512], BF16, kind="ExternalInput")
    wv_d = nc.dram_tensor("wv", [D, 512], BF16, kind="ExternalInput")
    wf_d = nc.dram_tensor("wf", [512, D], BF16, kind="ExternalInput")
    mb_d = nc.dram_tensor("mb", [128, KCT], F32, kind="ExternalInput")
    ones_d = nc.dram_tensor("ones", [128, 128], BF16, kind="ExternalInput")
    bq_d = nc.dram_tensor("bqt", [128, MT], F32, kind="ExternalInput")
    bk_d = nc.dram_tensor("bkt", [128, MT], F32, kind="ExternalInput")
    bv_d = nc.dram_tensor("bvr", [1, 512], BF16, kind="ExternalInput")
    npd_d = nc.dram_tensor("npd", [1, 1], F32, kind="ExternalInput")
    out_d = nc.dram_tensor("out", [QW, D], BF16, kind="ExternalOutput")

    with tile.TileContext(nc) as tc, nc.allow_low_precision(
        reason="bf16 matmul pipeline with fp32 accumulation"
    ), contextlib.ExitStack() as ctx:
        const = ctx.enter_context(tc.tile_pool(name="const", bufs=1))
        persist = ctx.enter_context(tc.tile_pool(name="persist", bufs=1))
        ppool = ctx.enter_context(tc.tile_pool(name="ppool", bufs=4))
        rpool = ctx.enter_context(tc.tile_pool(name="rpool", bufs=3))
        bcpool = ctx.enter_context(tc.tile_pool(name="bcpool", bufs=4))
        outpool = ctx.enter_context(tc.tile_pool(name="outpool", bufs=2))
        psum = ctx.enter_context(tc.tile_pool(name="psum", bufs=2, space="PSUM"))
        drpool = ctx.enter_context(tc.tile_pool(name="drpool", bufs=3,
                                                space="DRAM"))

        q_t = [persist.tile([128, QW], BF16, name=f"q_t{m}", tag=f"q_t{m}")
               for m in range(MT)]
        k_t = [persist.tile([128, KC], BF16, name=f"k_t{m}", tag=f"k_t{m}")
               for m in range(MT)]
        v65 = [persist.tile([128, HL * 65], BF16, name=f"v65_{t}",
                            tag=f"v65_{t}") for t in range(KCT)]
        o_t = [persist.tile([128, QW], BF16, name=f"o_t{p}", tag=f"o_t{p}")
               for p in range(NP)]
        xkv = [persist.tile([128, KC], BF16, name=f"xkv{d}", tag=f"xkv{d}")
               for d in range(DT)]
        xq = [persist.tile([128, QW], BF16, name=f"xq{d}", tag=f"xq{d}")
              for d in range(DT)]
        wv = [persist.tile([128, 512], BF16, name=f"wv{d}", tag=f"wv{d}")
              for d in range(DT)]
        wk = [persist.tile([128, 512], BF16, name=f"wk{d}", tag=f"wk{d}")
              for d in range(DT)]
        wq = [persist.tile([128, 512], BF16, name=f"wq{d}", tag=f"wq{d}")
              for d in range(DT)]
        wf = [persist.tile([128, D], BF16, name=f"wf{d}", tag=f"wf{d}")
              for d in range(MT)]

        # a3 deps stream first: first key-column block + wv unblock a3's
        # early tiles; the remainder and wk land under a3's compute
        for d in range(DT):
            nc.sync.dma_start(xkv[d][:, 0:512], xkv_d[ts(d, 128), 0:512])
            nc.sync.dma_start(wv[d][:], wv_d[ts(d, 128), :])
        for d in range(DT):
            nc.sync.dma_start(xkv[d][:, 512:KC], xkv_d[ts(d, 128), 512:KC])
        # consts off the critical sync queue
        mb_sb = const.tile([128, KCT], F32, name="mb_sb")
        npd_sb = const.tile([1, 1], F32, name="npd_sb")
        magic_sb = const.tile([128, 1024], F32, name="magic_sb")
        lnk_sb = const.tile([128, 1], F32, name="lnk_sb")
        if with_bias:
            nc.scalar.dma_start(mb_sb[:], mb_d[:])
        else:
            nc.scalar.dma_start(npd_sb[:], npd_d[:])
            nc.vector.memset(magic_sb[:], float(EXP2_MAGIC))
            nc.vector.memset(lnk_sb[:], float(LNKAPPA))
        exp2op = None if with_bias else _install_exp2()
        U16 = mybir.dt.uint16
        ones128 = const.tile([128, 128], BF16, name="ones128")
        bq_sb = const.tile([128, MT], F32, name="bq_sb")
        bk_sb = const.tile([128, MT], F32, name="bk_sb")
        bv_sb = const.tile([1, 512], BF16, name="bv_sb")
        if with_bias:
            nc.scalar.dma_start(ones128[:], ones_d[:])
            nc.scalar.dma_start(bq_sb[:], bq_d[:])
            nc.scalar.dma_start(bk_sb[:], bk_d[:])
            nc.scalar.dma_start(bv_sb[:], bv_d[:])
        for t in range(KCT):
            v3 = v65[t].rearrange("p (h e) -> p h e", e=65)
            nc.vector.memset(v3[:, :, 64:65], 1.0)

        # ---------------- phase pieces ----------------
        def a3(trange):
            for t in trange:
                pv = psum.tile([128, 512], F32, name=f"pv{t}", tag="ps1024")
                for d in range(DT):
                    nc.tensor.matmul(pv[:], xkv[d][:, ts(t, 128)], wv[d][:],
                                     start=(d == 0),
                                     stop=(not with_bias and d == DT - 1))
                if with_bias:
                    nc.tensor.matmul(pv[:], ones128[0:1, 0:128], bv_sb[0:1, :],
                                     start=False, stop=True)
                dst = v65[t].rearrange("p (h e) -> p h e", e=65)
                src = pv.rearrange("p (h e) -> p h e", e=64)
                nc.scalar.copy(dst[:, :, 0:64], src[:])

        def a2(mrange, scalar_evict=False):
            for m in mrange:
                pk = [psum.tile([128, w], F32, name=f"pk{m}_{c}", tag="ps1024")
                      for c, w in enumerate(kch)]
                for d in range(DT):
                    off = 0
                    for c, w in enumerate(kch):
                        nc.tensor.matmul(
                            pk[c][:], wk[d][:, ts(m, 128)],
                            xkv[d][:, off:off + w],
                            start=(d == 0), stop=(d == DT - 1))
                        off += w
                off = 0
                for c, w in enumerate(kch):
                    dst = k_t[m][:, off:off + w]
                    if with_bias:
                        nc.vector.tensor_scalar_add(dst, pk[c][:],
                                                    bk_sb[:, m:m + 1])
                    elif scalar_evict:
                        nc.scalar.copy(dst, pk[c][:])
                    else:
                        nc.vector.tensor_copy(dst, pk[c][:])
                    off += w

        def a1_half(m, h2, scalar_evict=False, tag="ps1024"):
            # generator: one [128, 1024] query-column half of q_t[m];
            # yields after every matmul so it can hide in B-phase slack
            pq = psum.tile([128, 1024], F32, name=f"pq{m}_{h2}", tag=tag,
                           bufs=2)
            for d in range(DT):
                for c in range(2):
                    nc.tensor.matmul(
                        pq[:, ts(c, 512)], wq[d][:, ts(m, 128)],
                        xq[d][:, h2 * 1024 + c * 512:
                              h2 * 1024 + (c + 1) * 512],
                        start=(d == 0), stop=(d == DT - 1))
                    yield
            dst = q_t[m][:, h2 * 1024:(h2 + 1) * 1024]
            if with_bias:
                nc.vector.tensor_scalar_add(dst, pq[:], bq_sb[:, m:m + 1])
            elif scalar_evict:
                nc.scalar.copy(dst, pq[:])
            else:
                nc.vector.tensor_copy(dst, pq[:])
            yield

        def c_tile(qt, scalar_evict=False, tag="ps1024"):
            # generator: one [128 q, 1024] FC output tile
            fp = psum.tile([128, 1024], F32, name=f"fp{qt}", tag=tag, bufs=2)
            for dt in range(MT):
                for c in range(2):
                    nc.tensor.matmul(fp[:, ts(c, 512)],
                                     o_t[dt][:, ts(qt, 128)],
                                     wf[dt][:, ts(c, 512)],
                                     start=(dt == 0), stop=(dt == MT - 1))
                    yield
            ost = outpool.tile([128, 1024], BF16, name=f"ost{qt}", tag="ost")
            if scalar_evict:
                nc.scalar.copy(ost[:], fp[:])
            else:
                nc.vector.tensor_copy(ost[:], fp[:])
            nc.sync.dma_start(out_d[ts(qt, 128), :], ost[:])
            yield

        def run_all(gen):
            for _ in gen:
                pass

        pending_norm = []

        def b_group(qc, p, filler=None, fill_per_step=1):
            # one head pair x KCT key tiles; PV lags S/exp by one tile
            oacc = psum.tile([128, 1024], F32, name=f"o2_{qc}_{p}",
                             tag="oB", bufs=2)
            h0, h1 = 2 * p, 2 * p + 1

            def emit_pv(kt, pp):
                nc.tensor.matmul(
                    oacc[0:65, 0:512], v65[kt][:, h0 * 65:(h0 + 1) * 65],
                    pp[:, 0:512],
                    start=(kt == 0), stop=(kt == KCT - 1))
                nc.tensor.matmul(
                    oacc[0:65, 512:1024], v65[kt][:, h1 * 65:(h1 + 1) * 65],
                    pp[:, 512:1024],
                    start=(kt == 0), stop=(kt == KCT - 1))

            # PV lags the S/exp stream by LAG key tiles: keeps ScalarE fed
            # and gives the previous group's accumulator time to evict
            LAG = 2
            pps = {}
            for kt in range(KCT):
                s = psum.tile([128, 1024], F32, name=f"s_{qc}_{p}_{kt}",
                              tag="ps1024")
                nc.tensor.matmul(s[:, 0:512], k_t[p][0:64, ts(kt, 128)],
                                 q_t[p][0:64, ts(qc, 512)])
                nc.tensor.matmul(s[:, 512:1024], k_t[p][64:128, ts(kt, 128)],
                                 q_t[p][64:128, ts(qc, 512)])
                pp = ppool.tile([128, 1024], BF16, name=f"pp_{qc}_{p}_{kt}",
                                tag="pp")
                if with_bias:
                    nc.scalar.activation(pp[:], s[:], AF.Exp,
                                         bias=mb_sb[:, kt:kt + 1],
                                         scale=float(SCALE))
                elif kt % 3 == 2:
                    nc.vector._custom_dve(
                        exp2op, out=pp.bitcast(U16), in0=s[:],
                        in1=magic_sb[:], s0=float(EXP2_C),
                        s1=float(EXP2_H), imm2=float(EXP2_A))
                else:
                    nc.scalar.activation(pp[:], s[:], AF.Exp,
                                         bias=lnk_sb[:, 0:1],
                                         scale=float(LN2_128))
                pps[kt] = pp
                if filler is not None:
                    for _ in range(fill_per_step):
                        try:
                            next(filler)
                        except StopIteration:
                            filler = None
                            break
                if kt >= LAG:
                    emit_pv(kt - LAG, pps.pop(kt - LAG))
            for kt in range(max(0, KCT - LAG), KCT):
                emit_pv(kt, pps.pop(kt))

            # stage denominator row + evacuate O^T (both heads)
            den = rpool.tile([1, 1024], F32, name=f"den_{qc}_{p}", tag="den",
                             bufs=3)
            if with_bias:
                nc.vector.tensor_copy(den[:], oacc[64:65, :])
            else:
                nc.vector.tensor_scalar_add(den[:], oacc[64:65, :],
                                            npd_sb[0:1, 0:1])
            qsl = ts(qc, 512)
            nc.vector.tensor_copy(o_t[p][0:64, qsl], oacc[0:64, 0:512])
            nc.vector.tensor_copy(o_t[p][64:128, qsl], oacc[0:64, 512:1024])
            pending_norm.append((qc, p, den))
            return filler

        def b_norm_late():
            # reciprocal on DVE; broadcast via DRAM bounce; normalization
            # multiplies on the otherwise-idle GpSimd
            while pending_norm:
                qc, p, den = pending_norm.pop(0)
                rec = rpool.tile([1, 1024], F32, name=f"rec_{qc}_{p}",
                                 tag="rec")
                nc.vector.reciprocal_approx_fast(rec[:], den[:])
                rdr = drpool.tile([1, 1024], F32, name=f"rdr_{qc}_{p}",
                                  tag="rdr")
                nc.sync.dma_start(rdr[:], rec[:])
                qsl = ts(qc, 512)
                bcs = bcpool.tile([128, 512], F32, name=f"bcs_{qc}_{p}",
                                  tag="bcs")
                nc.sync.dma_start(bcs[0:64, :],
                                  rdr[0:1, 0:512].to_broadcast([64, 512]))
                nc.sync.dma_start(bcs[64:128, :],
                                  rdr[0:1, 512:1024].to_broadcast([64, 512]))
                nc.gpsimd.tensor_mul(o_t[p][0:64, qsl],
                                     o_t[p][0:64, qsl], bcs[0:64, :])
                nc.gpsimd.tensor_mul(o_t[p][64:128, qsl],
                                     o_t[p][64:128, qsl], bcs[64:128, :])

        def chain(*gens):
            for g in gens:
                yield from g

        # ---------------- emission order ----------------
        a3(range(KCT))
        for d in range(DT):
            nc.sync.dma_start(wk[d][:], wk_d[ts(d, 128), :])
        a2(range(0, 2), scalar_evict=True)
        for d in range(DT):
            nc.sync.dma_start(xq[d][:], xq_d[ts(d, 128), :])
            nc.sync.dma_start(wq[d][:], wq_d[ts(d, 128), :])
        a2(range(2, 4), scalar_evict=True)
        for m in range(MT):
            run_all(a1_half(m, 0, scalar_evict=True))
        for m in range(MT):
            nc.sync.dma_start(wf[m][:], wf_d[ts(m, 128), :])

        # B: 16 groups (qc x pair). Fillers ride the exp-wall slack through
        # the spare oB slot. a1 h1 must complete before qc2 (group 9);
        # c tiles for qc K become legal after b_norm_late of qc K.
        fillers = {
            (0, 0): a1_half(0, 1, tag="oB", scalar_evict=True),
            (0, 2): a1_half(1, 1, tag="oB", scalar_evict=True),
            (1, 0): a1_half(2, 1, tag="oB", scalar_evict=True),
            (1, 2): a1_half(3, 1, tag="oB", scalar_evict=True),
            (2, 0): chain(c_tile(0, tag="oB", scalar_evict=True),
                          c_tile(1, tag="oB", scalar_evict=True)),
            (2, 2): chain(c_tile(2, tag="oB", scalar_evict=True),
                          c_tile(3, tag="oB", scalar_evict=True)),
            (3, 0): chain(c_tile(4, tag="oB", scalar_evict=True),
                          c_tile(5, tag="oB", scalar_evict=True)),
            (3, 2): chain(c_tile(6, tag="oB", scalar_evict=True),
                          c_tile(7, tag="oB", scalar_evict=True)),
        }
        cur = None
        for qc in range(4):
            for p in range(NP):
                if (qc, p) in fillers:
                    cur = fillers[(qc, p)]
                cur = b_group(qc, p, filler=cur, fill_per_step=2)
                if qc == 3:
                    # last qc: stream each pair's norm immediately so only
                    # the final pair's chain is exposed past the last group
                    b_norm_late()
            b_norm_late()
        for qt in range(8, 16):
            run_all(c_tile(qt, scalar_evict=True))

    nc.finalize()
    return nc


_LDW_PATCHED = False


def _enable_ldw_opt():
    global _LDW_PATCHED
    if _LDW_PATCHED:
        return
    import concourse.bass_utils as bu
    orig = bu.run_command

    def patched(cmd, *a, **k):
        cmd = [c.replace("--enable-ldw-opt=false", "--enable-ldw-opt=true")
               if isinstance(c, str) else c for c in cmd]
        return orig(cmd, *a, **k)

    bu.run_command = patched
    _LDW_PATCHED = True


def _get_program(KC, with_bias=True):
    key = (KC, with_bias)
    if key not in _programs:
        if __import__("os").environ.get("LDW_OPT"):
            _enable_ldw_opt()
        _programs[key] = _build(KC, with_bias)
    return _programs[key]


LAST_EXEC_NS = None
PROFILE = False


def _ensure_profile_hook():
    import types

    try:
        from antenv.axon_hooks import get_axon_ntff_profile_hook  # noqa: F401
        return
    except ImportError:
        pass
    import antenv

    mod = types.ModuleType("antenv.axon_hooks")
    _h = [None]
    mod.set_axon_ntff_profile_hook = lambda h: _h.__setitem__(0, h)
    mod.get_axon_ntff_profile_hook = lambda: _h[0]
    sys.modules["antenv.axon_hooks"] = mod
    antenv.axon_hooks = mod
    from trn_agent_boot.trn_boot import _ntff_profile_via_ctypes

    mod.set_axon_ntff_profile_hook(
        _ntff_profile_via_ctypes("/opt/axon/libaxon_pjrt.so"))
    import concourse.bass_utils as bu

    bu.upload_artifacts = lambda tmpdir: f"local:{tmpdir}"


def kernel(x, mask, Wq, bq, Wk, bk, Wv, bv, Wf, bf):
    global LAST_EXEC_NS
    from concourse.bass_utils import run_bass_kernel_spmd

    if PROFILE:
        _ensure_profile_hook()

    x = np.asarray(x, dtype=np.float32)
    mask = np.asarray(mask)
    keeps0 = [np.nonzero(np.asarray(mask[b]) == 0)[0] for b in range(BS)]
    wb0 = bool(np.any(np.asarray(bq)) or np.any(np.asarray(bk))
               or np.any(np.asarray(bv)))
    wk_scale = 1.0 if wb0 else ALPHA
    Wq16, Wk16, Wv16, Wf16 = (
        np.ascontiguousarray(np.asarray(w, np.float32) * s_).astype(BF16NP)
        for w, s_ in ((Wq, 1.0), (Wk, wk_scale), (Wv, 1.0), (Wf, 1.0)))
    bq32, bk32 = (np.asarray(v, np.float32) for v in (bq, bk))
    bv32 = np.asarray(bv, np.float32)
    bf32 = np.asarray(bf, np.float32)

    keeps = [np.nonzero(np.asarray(mask[b]) == 0)[0] for b in range(BS)]
    maxk = max(1, max(len(k) for k in keeps))
    KC = -(-maxk // 128) * 128
    with_bias = bool(np.any(bq32) or np.any(bk32) or np.any(bv32))
    nc = _get_program(KC, with_bias)
    KCT = KC // 128

    x16 = x.astype(BF16NP)
    in_maps = []
    for c in range(8):
        b, j = divmod(c, 2)
        keep = keeps[b]
        xq_t = np.ascontiguousarray(x16[b].T)
        xkv_t = np.zeros((D, KC), BF16NP)
        xkv_t[:, :len(keep)] = x16[b, keep, :].T
        mbv = np.full(KC, NEG, np.float32)
        mbv[:len(keep)] = 0.0
        mb_t = np.ascontiguousarray(mbv.reshape(KCT, 128).T)
        sl = slice(512 * j, 512 * (j + 1))
        in_maps.append({
            "ones": _ONES, "xq": xq_t, "xkv": xkv_t,
            "npd": np.full((1, 1), -float(KC - len(keep))
                           * EXP2_KAPPA, np.float32),
            "wq": np.ascontiguousarray(Wq16[:, sl]),
            "wk": np.ascontiguousarray(Wk16[:, sl]),
            "wv": np.ascontiguousarray(Wv16[:, sl]),
            "wf": np.ascontiguousarray(Wf16[sl, :]),
            "mb": mb_t,
            "bqt": np.ascontiguousarray(bq32[sl].reshape(MT, 128).T),
            "bkt": np.ascontiguousarray(bk32[sl].reshape(MT, 128).T),
            "bvr": bv32[sl].astype(BF16NP).reshape(1, 512),
        })

    res = run_bass_kernel_spmd(nc, in_maps, core_ids=list(range(8)),
                               trace=PROFILE)
    if res.exec_time_ns is not None:
        LAST_EXEC_NS = res.exec_time_ns

    out = np.empty((BS, L, D), np.float32)
    for b in range(BS):
        out[b] = (res.results[2 * b]["out"].astype(np.float32)
                  + res.results[2 * b + 1]["out"].astype(np.float32))
    out += bf32.reshape(1, 1, D)
    return out


# revision 6
# speedup vs baseline: 1.0328x; 1.0065x over previous
"""Trainium2 Bass kernel for nn_MultiHeadAttention (BS=4, L=2048, D=1024, H=16).

Sharding: 8 cores = 4 batches x 2 head-halves. Core (b, j) computes heads
8j..8j+8 of batch b for ALL 2048 queries, K/V over the batch's unmasked keys
(host gather). Wq/Wk/Wv column-sharded, Wf row-sharded; each core emits a
bf16 PARTIAL out = O_local @ Wf_rows and the host adds the two partials per
batch (+ bf).

Schedule: a3 (V), a2 (K), a1-half (Q cols 0:1024) run upfront, PE-dense.
The B phase processes one head-pair per group (16 groups of 9 key tiles);
it is exp-wall-bound on ScalarE (one ACTIVATE per key tile). PV matmuls lag
the S/exp stream by one key tile (software pipelining) so ScalarE never
starves, and the PE slack under the exp wall is filled by injecting the
remaining a1 matmuls and the first 8 FC (C) tiles one matmul per step
through the spare oB PSUM slot (one pair per group leaves one of the two oB
slots free to rotate through filler tiles). Denominator staging is a small
Vector copy; o_t normalization multiplies run on the otherwise-idle GpSimd
so the Vector FIFO cannot block PSUM evictions.
"""
import sys

sys.path.insert(0, "/opt/trn_rl_repo")

import numpy as np
import ml_dtypes

BF16NP = ml_dtypes.bfloat16

BS, L, D, H, DK = 4, 2048, 1024, 16, 64
SCALE = 1.0 / np.sqrt(DK)
QW = L             # queries per core
HL = 8             # local heads
NP = HL // 2       # local head pairs
MT = 4             # local out-dim tiles (512/128)
DT = D // 128      # contraction d tiles
NEG = -1.0e30

_programs = {}
_ONES = np.ones((128, 128), BF16NP)


def _chunks(total, maxc=512):
    n = -(-total // maxc)
    base = total // n
    rem = total - base * n
    return [base + (1 if i < rem else 0) for i in range(n)]




# ---- custom-DVE exp2: bf16-bits construction in one 8-stage DVE op ----
# value(Z-bits) = KAPPA * 2^(y/128) with |rel err| <= 0.0054 (incl. bf16
# rounding) for |y| <= 1100. The K projection is host-prescaled by ALPHA so
# the PSUM logit arrives as y = 128*log2(e)*SCALE*s_raw; ScalarE kts match
# via exp(y*ln2/128 + ln KAPPA).
EXP2_MAGIC = 1.5 * 2**30 + 16128.0
EXP2_H = 186.0380113
EXP2_A = 0.0026865429
EXP2_C = 16088.5532310
EXP2_KAPPA = 0.7088638733
ALPHA = 128.0 * np.log2(np.e) * (1.0 / np.sqrt(DK))
LN2_128 = np.log(2.0) / 128.0
LNKAPPA = float(np.log(EXP2_KAPPA))

_EXP2_OP = None


def _install_exp2():
    global _EXP2_OP
    if _EXP2_OP is not None:
        return _EXP2_OP
    import concourse.dve_ops as dops
    import concourse.dve_spec as dspec
    from concourse.dve_spec import Spec, Src0, Src1, C0, C1, C2
    from concourse.dve_uop import DveOpSpec

    name = "EXP2_BITS_ANT"
    T = Src0 + Src1
    N = T - Src1
    R = Src0 - N
    u = R + C1
    Z = N + ((u * u) * C2 + C0)

    def ref(in0, in1, c0, c1, c2):
        S = np.asarray(in0, np.float32)
        M = np.asarray(in1, np.float32)
        Tv = (S + M).astype(np.float32)
        Nv = (Tv - M).astype(np.float32)
        Rv = (S - Nv).astype(np.float32)
        uv = (Rv + np.float32(c1)).astype(np.float32)
        Zv = (Nv + (uv * uv * np.float32(c2) + np.float32(c0))).astype(
            np.float32)
        return np.clip(np.rint(Zv), 0, 65535)

    spec = Spec(body=Z, reference=ref)
    if name not in dops._SUB_OPCODE_FOR_NAME:
        row = max(dops._SUB_OPCODE_FOR_NAME.values()) + 1
        assert row < 0x20
        dops._SUB_OPCODE_FOR_NAME[name] = row
    row = dops._SUB_OPCODE_FOR_NAME[name]
    from concourse.dve_table_gen import dve_ver_for as _vf
    try:
        ver = _vf("TRN2")
    except Exception:
        ver = "v3"
    uops = dspec.lower(spec, ver=ver)
    sha = DveOpSpec(name=name, opcode=row, uops=uops,
                    rd1_en=dspec._has_src1(spec)).sha(ver)
    op = dops.DveOp(name, spec, subdim=False, uops_sha={ver: sha})
    if all(o.name != name for o in dops.OPS):
        dops.OPS.append(op)
    dops.CUSTOM_DVE_SPECS[name] = spec
    _EXP2_OP = op
    return op


def _build(KC, with_bias=True):
    import contextlib

    import concourse.bacc as bacc
    import concourse.bass as bass
    import concourse.tile as tile
    import concourse.mybir as mybir

    F32 = mybir.dt.float32
    BF16 = mybir.dt.bfloat16
    AF = mybir.ActivationFunctionType
    ts = bass.ts

    KCT = KC // 128
    kch = _chunks(KC)

    nc = bacc.Bacc("TRN2", target_bir_lowering=False)

    xq_d = nc.dram_tensor("xq", [D, QW], BF16, kind="ExternalInput")
    xkv_d = nc.dram_tensor("xkv", [D, KC], BF16, kind="ExternalInput")
    wq_d = nc.dram_tensor("wq", [D, 512], BF16, kind="ExternalInput")
    wk_d = nc.dram_tensor("wk", [D, 512], BF16, kind="ExternalInput")
    wv_d = nc.dram_tensor("wv", [D, 512], BF16, kind="ExternalInput")
    wf_d = nc.dram_tensor("wf", [512, D], BF16, kind="ExternalInput")
    mb_d = nc.dram_tensor("mb", [128, KCT], F32, kind="ExternalInput")
    ones_d = nc.dram_tensor("ones", [128, 128], BF16, kind="ExternalInput")
    bq_d = nc.dram_tensor("bqt", [128, MT], F32, kind="ExternalInput")
    bk_d = nc.dram_tensor("bkt", [128, MT], F32, kind="ExternalInput")
    bv_d = nc.dram_tensor("bvr", [1, 512], BF16, kind="ExternalInput")
    npd_d = nc.dram_tensor("npd", [1, 1], F32, kind="ExternalInput")
    out_d = nc.dram_tensor("out", [QW, D], BF16, kind="ExternalOutput")

    with tile.TileContext(nc) as tc, nc.allow_low_precision(
        reason="bf16 matmul pipeline with fp32 accumulation"
    ), contextlib.ExitStack() as ctx:
        const = ctx.enter_context(tc.tile_pool(name="const", bufs=1))
        persist = ctx.enter_context(tc.tile_pool(name="persist", bufs=1))
        ppool = ctx.enter_context(tc.tile_pool(name="ppool", bufs=6))
        rpool = ctx.enter_context(tc.tile_pool(name="rpool", bufs=3))
        bcpool = ctx.enter_context(tc.tile_pool(name="bcpool", bufs=4))
        outpool = ctx.enter_context(tc.tile_pool(name="outpool", bufs=2))
        psum = ctx.enter_context(tc.tile_pool(name="psum", bufs=2, space="PSUM"))
        drpool = ctx.enter_context(tc.tile_pool(name="drpool", bufs=3,
                                                space="DRAM"))

        q_t = [persist.tile([128, QW], BF16, name=f"q_t{m}", tag=f"q_t{m}")
               for m in range(MT)]
        k_t = [persist.tile([128, KC], BF16, name=f"k_t{m}", tag=f"k_t{m}")
               for m in range(MT)]
        v65 = [persist.tile([128, HL * 65], BF16, name=f"v65_{t}",
                            tag=f"v65_{t}") for t in range(KCT)]
        o_t = [persist.tile([128, QW], BF16, name=f"o_t{p}", tag=f"o_t{p}")
               for p in range(NP)]
        xkv = [persist.tile([128, KC], BF16, name=f"xkv{d}", tag=f"xkv{d}")
               for d in range(DT)]
        xq = [persist.tile([128, QW], BF16, name=f"xq{d}", tag=f"xq{d}")
              for d in range(DT)]
        wv = [persist.tile([128, 512], BF16, name=f"wv{d}", tag=f"wv{d}")
              for d in range(DT)]
        wk = [persist.tile([128, 512], BF16, name=f"wk{d}", tag=f"wk{d}")
              for d in range(DT)]
        wq = [persist.tile([128, 512], BF16, name=f"wq{d}", tag=f"wq{d}")
              for d in range(DT)]
        wf = [persist.tile([128, D], BF16, name=f"wf{d}", tag=f"wf{d}")
              for d in range(MT)]

        # a3 deps stream first: first key-column block + wv unblock a3's
        # early tiles; the remainder and wk land under a3's compute
        for d in range(DT):
            nc.sync.dma_start(xkv[d][:, 0:512], xkv_d[ts(d, 128), 0:512])
            nc.sync.dma_start(wv[d][:], wv_d[ts(d, 128), :])
        for d in range(DT):
            nc.sync.dma_start(xkv[d][:, 512:KC], xkv_d[ts(d, 128), 512:KC])
        # consts off the critical sync queue
        mb_sb = const.tile([128, KCT], F32, name="mb_sb")
        npd_sb = const.tile([1, 1], F32, name="npd_sb")
        magic_sb = const.tile([128, 1024], F32, name="magic_sb")
        lnk_sb = const.tile([128, 1], F32, name="lnk_sb")
        if with_bias:
            nc.scalar.dma_start(mb_sb[:], mb_d[:])
        else:
            nc.scalar.dma_start(npd_sb[:], npd_d[:])
            nc.vector.memset(magic_sb[:], float(EXP2_MAGIC))
            nc.vector.memset(lnk_sb[:], float(LNKAPPA))
        exp2op = None if with_bias else _install_exp2()
        U16 = mybir.dt.uint16
        ones128 = const.tile([128, 128], BF16, name="ones128")
        bq_sb = const.tile([128, MT], F32, name="bq_sb")
        bk_sb = const.tile([128, MT], F32, name="bk_sb")
        bv_sb = const.tile([1, 512], BF16, name="bv_sb")
        if with_bias:
            nc.scalar.dma_start(ones128[:], ones_d[:])
            nc.scalar.dma_start(bq_sb[:], bq_d[:])
            nc.scalar.dma_start(bk_sb[:], bk_d[:])
            nc.scalar.dma_start(bv_sb[:], bv_d[:])
        for t in range(KCT):
            v3 = v65[t].rearrange("p (h e) -> p h e", e=65)
            nc.vector.memset(v3[:, :, 64:65], 1.0)

        # ---------------- phase pieces ----------------
        def a3(trange):
            for t in trange:
                pv = psum.tile([128, 512], F32, name=f"pv{t}", tag="ps1024")
                for d in range(DT):
                    nc.tensor.matmul(pv[:], xkv[d][:, ts(t, 128)], wv[d][:],
                                     start=(d == 0),
                                     stop=(not with_bias and d == DT - 1))
                if with_bias:
                    nc.tensor.matmul(pv[:], ones128[0:1, 0:128], bv_sb[0:1, :],
                                     start=False, stop=True)
                dst = v65[t].rearrange("p (h e) -> p h e", e=65)
                src = pv.rearrange("p (h e) -> p h e", e=64)
                nc.scalar.copy(dst[:, :, 0:64], src[:])

        def a2(mrange, scalar_evict=False):
            for m in mrange:
                pk = [psum.tile([128, w], F32, name=f"pk{m}_{c}", tag="ps1024")
                      for c, w in enumerate(kch)]
                for d in range(DT):
                    off = 0
                    for c, w in enumerate(kch):
                        nc.tensor.matmul(
                            pk[c][:], wk[d][:, ts(m, 128)],
                            xkv[d][:, off:off + w],
                            start=(d == 0), stop=(d == DT - 1))
                        off += w
                off = 0
                for c, w in enumerate(kch):
                    dst = k_t[m][:, off:off + w]
                    if with_bias:
                        nc.vector.tensor_scalar_add(dst, pk[c][:],
                                                    bk_sb[:, m:m + 1])
                    elif scalar_evict:
                        nc.scalar.copy(dst, pk[c][:])
                    else:
                        nc.vector.tensor_copy(dst, pk[c][:])
                    off += w

        def a1_half(m, h2, scalar_evict=False, tag="ps1024"):
            # generator: one [128, 1024] query-column half of q_t[m];
            # yields after every matmul so it can hide in B-phase slack
            pq = psum.tile([128, 1024], F32, name=f"pq{m}_{h2}", tag=tag,
                           bufs=2)
            for d in range(DT):
                for c in range(2):
                    nc.tensor.matmul(
                        pq[:, ts(c, 512)], wq[d][:, ts(m, 128)],
                        xq[d][:, h2 * 1024 + c * 512:
                              h2 * 1024 + (c + 1) * 512],
                        start=(d == 0), stop=(d == DT - 1))
                    yield
            dst = q_t[m][:, h2 * 1024:(h2 + 1) * 1024]
            if with_bias:
                nc.vector.tensor_scalar_add(dst, pq[:], bq_sb[:, m:m + 1])
            elif scalar_evict:
                nc.scalar.copy(dst, pq[:])
            else:
                nc.vector.tensor_copy(dst, pq[:])
            yield

        def c_tile(qt, scalar_evict=False, tag="ps1024"):
            # generator: one [128 q, 1024] FC output tile
            fp = psum.tile([128, 1024], F32, name=f"fp{qt}", tag=tag, bufs=2)
            for dt in range(MT):
                for c in range(2):
                    nc.tensor.matmul(fp[:, ts(c, 512)],
                                     o_t[dt][:, ts(qt, 128)],
                                     wf[dt][:, ts(c, 512)],
                                     start=(dt == 0), stop=(dt == MT - 1))
                    yield
            ost = outpool.tile([128, 1024], BF16, name=f"ost{qt}", tag="ost")
            if scalar_evict:
                nc.scalar.copy(ost[:], fp[:])
            else:
                nc.vector.tensor_copy(ost[:], fp[:])
            nc.sync.dma_start(out_d[ts(qt, 128), :], ost[:])
            yield

        def run_all(gen):
            for _ in gen:
                pass

        pending_norm = []

        def b_group(qc, p, filler=None, fill_per_step=1):
            # one head pair x KCT key tiles; PV lags S/exp by one tile
            oacc = psum.tile([128, 1024], F32, name=f"o2_{qc}_{p}",
                             tag="oB", bufs=2)
            h0, h1 = 2 * p, 2 * p + 1

            def emit_pv(kt, pp):
                nc.tensor.matmul(
                    oacc[0:65, 0:512], v65[kt][:, h0 * 65:(h0 + 1) * 65],
                    pp[:, 0:512],
                    start=(kt == 0), stop=(kt == KCT - 1))
                nc.tensor.matmul(
                    oacc[0:65, 512:1024], v65[kt][:, h1 * 65:(h1 + 1) * 65],
                    pp[:, 512:1024],
                    start=(kt == 0), stop=(kt == KCT - 1))

            # PV lags the S/exp stream by LAG key tiles: keeps ScalarE fed
            # and gives the previous group's accumulator time to evict
            LAG = 2
            pps = {}
            for kt in range(KCT):
                s = psum.tile([128, 1024], F32, name=f"s_{qc}_{p}_{kt}",
                              tag="ps1024")
                nc.tensor.matmul(s[:, 0:512], k_t[p][0:64, ts(kt, 128)],
                                 q_t[p][0:64, ts(qc, 512)])
                nc.tensor.matmul(s[:, 512:1024], k_t[p][64:128, ts(kt, 128)],
                                 q_t[p][64:128, ts(qc, 512)])
                pp = ppool.tile([128, 1024], BF16, name=f"pp_{qc}_{p}_{kt}",
                                tag="pp")
                if with_bias:
                    nc.scalar.activation(pp[:], s[:], AF.Exp,
                                         bias=mb_sb[:, kt:kt + 1],
                                         scale=float(SCALE))
                elif kt % 3 == 2:
                    nc.vector._custom_dve(
                        exp2op, out=pp.bitcast(U16), in0=s[:],
                        in1=magic_sb[:], s0=float(EXP2_C),
                        s1=float(EXP2_H), imm2=float(EXP2_A))
                else:
                    nc.scalar.activation(pp[:], s[:], AF.Exp,
                                         bias=lnk_sb[:, 0:1],
                                         scale=float(LN2_128))
                pps[kt] = pp
                if filler is not None:
                    for _ in range(fill_per_step):
                        try:
                            next(filler)
                        except StopIteration:
                            filler = None
                            break
                if kt >= LAG:
                    emit_pv(kt - LAG, pps.pop(kt - LAG))
            for kt in range(max(0, KCT - LAG), KCT):
                emit_pv(kt, pps.pop(kt))

            # stage denominator row + evacuate O^T (both heads)
            den = rpool.tile([1, 1024], F32, name=f"den_{qc}_{p}", tag="den",
                             bufs=3)
            if with_bias:
                nc.vector.tensor_copy(den[:], oacc[64:65, :])
            else:
                nc.vector.tensor_scalar_add(den[:], oacc[64:65, :],
                                            npd_sb[0:1, 0:1])
            qsl = ts(qc, 512)
            nc.vector.tensor_copy(o_t[p][0:64, qsl], oacc[0:64, 0:512])
            nc.vector.tensor_copy(o_t[p][64:128, qsl], oacc[0:64, 512:1024])
            pending_norm.append((qc, p, den))
            return filler

        def b_norm_late():
            # reciprocal on DVE; broadcast via DRAM bounce; normalization
            # multiplies on the otherwise-idle GpSimd
            while pending_norm:
                qc, p, den = pending_norm.pop(0)
                rec = rpool.tile([1, 1024], F32, name=f"rec_{qc}_{p}",
                                 tag="rec")
                nc.vector.reciprocal_approx_fast(rec[:], den[:])
                rdr = drpool.tile([1, 1024], F32, name=f"rdr_{qc}_{p}",
                                  tag="rdr")
                nc.sync.dma_start(rdr[:], rec[:])
                qsl = ts(qc, 512)
                bcs = bcpool.tile([128, 512], F32, name=f"bcs_{qc}_{p}",
                                  tag="bcs")
                nc.sync.dma_start(bcs[0:64, :],
                                  rdr[0:1, 0:512].to_broadcast([64, 512]))
                nc.sync.dma_start(bcs[64:128, :],
                                  rdr[0:1, 512:1024].to_broadcast([64, 512]))
                nc.gpsimd.tensor_mul(o_t[p][0:64, qsl],
                                     o_t[p][0:64, qsl], bcs[0:64, :])
                nc.gpsimd.tensor_mul(o_t[p][64:128, qsl],
                                     o_t[p][64:128, qsl], bcs[64:128, :])

        def chain(*gens):
            for g in gens:
                yield from g

        # ---------------- emission order ----------------
        a3(range(KCT))
        for d in range(DT):
            nc.sync.dma_start(wk[d][:], wk_d[ts(d, 128), :])
        a2(range(0, 2), scalar_evict=True)
        for d in range(DT):
            nc.sync.dma_start(xq[d][:], xq_d[ts(d, 128), :])
            nc.sync.dma_start(wq[d][:], wq_d[ts(d, 128), :])
        a2(range(2, 4), scalar_evict=True)
        for m in range(MT):
            run_all(a1_half(m, 0, scalar_evict=True))
        for m in range(MT):
            nc.sync.dma_start(wf[m][:], wf_d[ts(m, 128), :])

        # B: 16 groups (qc x pair). Fillers ride the exp-wall slack through
        # the spare oB slot. a1 h1 must complete before qc2 (group 9);
        # c tiles for qc K become legal after b_norm_late of qc K.
        fillers = {
            (0, 0): a1_half(0, 1, tag="oB", scalar_evict=True),
            (0, 2): a1_half(1, 1, tag="oB", scalar_evict=True),
            (1, 0): a1_half(2, 1, tag="oB", scalar_evict=True),
            (1, 2): a1_half(3, 1, tag="oB", scalar_evict=True),
            (2, 0): chain(c_tile(0, tag="oB", scalar_evict=True),
                          c_tile(1, tag="oB", scalar_evict=True)),
            (2, 2): chain(c_tile(2, tag="oB", scalar_evict=True),
                          c_tile(3, tag="oB", scalar_evict=True)),
            (3, 0): chain(c_tile(4, tag="oB", scalar_evict=True),
                          c_tile(5, tag="oB", scalar_evict=True)),
            (3, 1): c_tile(8, tag="oB", scalar_evict=True),
            (3, 2): chain(c_tile(6, tag="oB", scalar_evict=True),
                          c_tile(7, tag="oB", scalar_evict=True)),
            (3, 3): c_tile(9, tag="oB", scalar_evict=True),
        }
        cur = None
        for qc in range(4):
            for p in range(NP):
                if (qc, p) in fillers:
                    cur = fillers[(qc, p)]
                cur = b_group(qc, p, filler=cur, fill_per_step=2)
                if qc == 3:
                    # last qc: stream each pair's norm immediately so only
                    # the final pair's chain is exposed past the last group
                    b_norm_late()
            b_norm_late()
        for qt in range(10, 16):
            run_all(c_tile(qt, scalar_evict=True))

    nc.finalize()
    return nc


_LDW_PATCHED = False


def _enable_ldw_opt():
    global _LDW_PATCHED
    if _LDW_PATCHED:
        return
    import concourse.bass_utils as bu
    orig = bu.run_command

    def patched(cmd, *a, **k):
        cmd = [c.replace("--enable-ldw-opt=false", "--enable-ldw-opt=true")
               if isinstance(c, str) else c for c in cmd]
        return orig(cmd, *a, **k)

    bu.run_command = patched
    _LDW_PATCHED = True


def _get_program(KC, with_bias=True):
    key = (KC, with_bias)
    if key not in _programs:
        if __import__("os").environ.get("LDW_OPT"):
            _enable_ldw_opt()
        _programs[key] = _build(KC, with_bias)
    return _programs[key]


LAST_EXEC_NS = None
PROFILE = False


def _ensure_profile_hook():
    import types

    try:
        from antenv.axon_hooks import get_axon_ntff_profile_hook  # noqa: F401
        return
    except ImportError:
        pass
    import antenv

    mod = types.ModuleType("antenv.axon_hooks")
    _h = [None]
    mod.set_axon_ntff_profile_hook = lambda h: _h.__setitem__(0, h)
    mod.get_axon_ntff_profile_hook = lambda: _h[0]
    sys.modules["antenv.axon_hooks"] = mod
    antenv.axon_hooks = mod
    from trn_agent_boot.trn_boot import _ntff_profile_via_ctypes

    mod.set_axon_ntff_profile_hook(
        _ntff_profile_via_ctypes("/opt/axon/libaxon_pjrt.so"))
    import concourse.bass_utils as bu

    bu.upload_artifacts = lambda tmpdir: f"local:{tmpdir}"


def kernel(x, mask, Wq, bq, Wk, bk, Wv, bv, Wf, bf):
    global LAST_EXEC_NS
    from concourse.bass_utils import run_bass_kernel_spmd

    if PROFILE:
        _ensure_profile_hook()

    x = np.asarray(x, dtype=np.float32)
    mask = np.asarray(mask)
    keeps0 = [np.nonzero(np.asarray(mask[b]) == 0)[0] for b in range(BS)]
    wb0 = bool(np.any(np.asarray(bq)) or np.any(np.asarray(bk))
               or np.any(np.asarray(bv)))
    wk_scale = 1.0 if wb0 else ALPHA
    Wq16, Wk16, Wv16, Wf16 = (
        np.ascontiguousarray(np.asarray(w, np.float32) * s_).astype(BF16NP)
        for w, s_ in ((Wq, 1.0), (Wk, wk_scale), (Wv, 1.0), (Wf, 1.0)))
    bq32, bk32 = (np.asarray(v, np.float32) for v in (bq, bk))
    bv32 = np.asarray(bv, np.float32)
    bf32 = np.asarray(bf, np.float32)

    keeps = [np.nonzero(np.asarray(mask[b]) == 0)[0] for b in range(BS)]
    maxk = max(1, max(len(k) for k in keeps))
    KC = -(-maxk // 128) * 128
    with_bias = bool(np.any(bq32) or np.any(bk32) or np.any(bv32))
    nc = _get_program(KC, with_bias)
    KCT = KC // 128

    x16 = x.astype(BF16NP)
    in_maps = []
    for c in range(8):
        b, j = divmod(c, 2)
        keep = keeps[b]
        xq_t = np.ascontiguousarray(x16[b].T)
        xkv_t = np.zeros((D, KC), BF16NP)
        xkv_t[:, :len(keep)] = x16[b, keep, :].T
        mbv = np.full(KC, NEG, np.float32)
        mbv[:len(keep)] = 0.0
        mb_t = np.ascontiguousarray(mbv.reshape(KCT, 128).T)
        sl = slice(512 * j, 512 * (j + 1))
        in_maps.append({
            "ones": _ONES, "xq": xq_t, "xkv": xkv_t,
            "npd": np.full((1, 1), -float(KC - len(keep))
                           * EXP2_KAPPA, np.float32),
            "wq": np.ascontiguousarray(Wq16[:, sl]),
            "wk": np.ascontiguousarray(Wk16[:, sl]),
            "wv": np.ascontiguousarray(Wv16[:, sl]),
            "wf": np.ascontiguousarray(Wf16[sl, :]),
            "mb": mb_t,
            "bqt": np.ascontiguousarray(bq32[sl].reshape(MT, 128).T),
            "bkt": np.ascontiguousarray(bk32[sl].reshape(MT, 128).T),
            "bvr": bv32[sl].astype(BF16NP).reshape(1, 512),
        })

    res = run_bass_kernel_spmd(nc, in_maps, core_ids=list(range(8)),
                               trace=PROFILE)
    if res.exec_time_ns is not None:
        LAST_EXEC_NS = res.exec_time_ns

    out = np.empty((BS, L, D), np.float32)
    for b in range(BS):
        out[b] = (res.results[2 * b]["out"].astype(np.float32)
                  + res.results[2 * b + 1]["out"].astype(np.float32))
    out += bf32.reshape(1, 1, D)
    return out
